# revision 50
# baseline (speedup 1.0000x reference)
"""Multi-head latent attention (MLA) forward pass on 8 Trainium2 NeuronCores.

Sharding: 2 (batch) x 4 (head-group) grid. Core c handles batch b = c // 4
and heads 4*(c % 4) .. 4*(c % 4) + 3.  Per core:
  - streams x[b]^T (host-pretransposed, bf16) once
  - kv_down GEMM in TRANSPOSED form: ckv^T[lat, s] = Wkd_chunk^T.T @ x^T,
    rms-norm over lat via ones-matmul (partition reduce) + gpsimd broadcast;
    born-transposed => no PE transposes for the latent.
  - GEMM-A: A[s, 768] = x_s @ [Wq_heads | Wkrope_heads]; batched rms-norms;
    rope applied in-place; ONE 128-wide PE transpose per head for q
    (nope||rope together) into QT.
  - GEMM-3 (kv_up) fused per s-tile (uses just-computed ckv^T columns);
    k_nope normed into kbuf[0:64], k_rope (phase A) in kbuf[64:128] =>
    ONE 128-wide transpose per head for k into KT.
  - causal attention per head in transposed-score form; softmax denominator
    via DVE-accumulated P (groups of 4 k-tiles) + one ones-matmul per group
    instead of one per k-tile.
  - out projection fused per q-block j; partials DMA'd as computed.
Host sums the 4 partials per batch element.
"""

import sys

for _p in ("/opt/trn_rl_repo",):
    if _p not in sys.path:
        sys.path.insert(0, _p)

import math
from contextlib import ExitStack

import ml_dtypes
import numpy as np

import concourse.bass as bass
import concourse.bass_isa as bass_isa
import concourse.mybir as mybir
import concourse.tile as tile
from concourse import bacc
from concourse.bass_utils import run_bass_kernel_spmd

F32 = mybir.dt.float32
BF16 = mybir.dt.bfloat16
BF = ml_dtypes.bfloat16

B, S, D = 2, 2048, 2048
H = 16
HD = 128           # head dim
ROPE = 64
NOPE = 64
LAT = 512
EPS = 1e-6
ROPE_BASE = 10000.0

H_LOC = 4          # heads per core
N_CORES = 8
DLOC = H_LOC * HD  # 512, per-core proj contraction size

ST_N = S // 128    # 16 s-tiles
KT_N = D // 128    # 16 k-tiles for GEMM-A
QB = 512           # attention q-block width
NB = 512           # proj output block width
LGRP = 4           # k-tiles per softmax-denominator accumulation group

A_QW = H_LOC * HD             # 512  q columns in A
A_RW = H_LOC * ROPE           # 256  k_rope columns in A
A_W = A_QW + A_RW             # 768 total A columns (kv_down separate)
KV_W = H_LOC * NOPE + H_LOC * HD   # 768 kv columns
LT_N = LAT // 128  # 4 latent tiles

X8_CHUNK = 256                # s-columns of x^T per streamed chunk
X8_N = S // X8_CHUNK          # 8 chunks
ST_PER_CHUNK = X8_CHUNK // 128  # 2

MULT = mybir.AluOpType.mult
ADD = mybir.AluOpType.add
SUB = mybir.AluOpType.subtract
EXPF = mybir.ActivationFunctionType.Exp
SQRTF = mybir.ActivationFunctionType.Sqrt
SQF = mybir.ActivationFunctionType.Square
AXX = mybir.AxisListType.X

_PROGRAM_CACHE = {}


def _build_program(debug_taps=False):
    nc = bacc.Bacc(None, target_bir_lowering=False, debug=True)

    # ---- DRAM I/O ----
    xT8 = nc.dram_tensor("xT8", [X8_N, D, X8_CHUNK], BF16, kind="ExternalInput")
    xkv = nc.dram_tensor("xkv", [2, D, X8_CHUNK], BF16, kind="ExternalInput")
    w_a = nc.dram_tensor("w_a", [D, A_W], BF16, kind="ExternalInput")
    w_kd = nc.dram_tensor("w_kd", [D, LAT], BF16, kind="ExternalInput")
    w_up = nc.dram_tensor("w_up", [LAT, KV_W], BF16, kind="ExternalInput")
    w_p = nc.dram_tensor("w_p", [DLOC, D], BF16, kind="ExternalInput")
    cos4 = nc.dram_tensor("cos4", [S, H_LOC, ROPE // 2], BF16, kind="ExternalInput")
    sin4 = nc.dram_tensor("sin4", [S, H_LOC, ROPE // 2], BF16, kind="ExternalInput")
    masks = nc.dram_tensor("masks", [128, 4, QB], BF16, kind="ExternalInput")
    gain12 = nc.dram_tensor("gain12", [128, 12], F32, kind="ExternalInput")
    ones_in = nc.dram_tensor("ones_in", [128, 1], BF16, kind="ExternalInput")
    ident_in = nc.dram_tensor("ident_in", [128, 128], BF16, kind="ExternalInput")
    out = nc.dram_tensor("out", [S, D], BF16, kind="ExternalOutput")
    if debug_taps:
        dbg_ckvT = nc.dram_tensor("dbg_ckvT", [128, LT_N, S], BF16, kind="ExternalOutput")
        dbg_QT = nc.dram_tensor("dbg_QT", [128, H_LOC, S], BF16, kind="ExternalOutput")
        dbg_KT = nc.dram_tensor("dbg_KT", [128, H_LOC, S], BF16, kind="ExternalOutput")
        dbg_V = nc.dram_tensor("dbg_V", [128, ST_N, H_LOC * HD], BF16, kind="ExternalOutput")

    with tile.TileContext(nc) as tc, ExitStack() as top:
        const = top.enter_context(tc.tile_pool(name="const", bufs=1))
        big = top.enter_context(tc.tile_pool(name="big", bufs=1))

        # --- resident weights/constants, load order = consumption order ---
        wkd_sbs = [const.tile([128, 4, LT_N, 128], BF16, name=f"wkd{i}")
                   for i in range(4)]
        wkd_r = w_kd[:].rearrange("(k p) (l q) -> p k l q", p=128, q=128)
        wa_sb = const.tile([128, KT_N, A_W], BF16)
        wa_r = w_a[:].rearrange("(k p) n -> p k n", p=128)
        wup_sb = const.tile([128, LT_N, KV_W], BF16)
        cos_sb = const.tile([128, ST_N, H_LOC, ROPE // 2], BF16)
        sin_sb = const.tile([128, ST_N, H_LOC, ROPE // 2], BF16)
        gain_sb = const.tile([128, 12], F32)
        ones_sb = const.tile([128, 1], BF16)
        ident_sb = const.tile([128, 128], BF16)
        eps_sb = const.tile([128, 1], F32)
        # loaded late (only needed by attention/proj phase)
        mask_sb = const.tile([128, 4, QB], BF16)
        wp_sb = const.tile([128, H_LOC, D], BF16)

        # --- persistent activations (head-dim-major) ---
        QT = big.tile([128, H_LOC, S], BF16)   # [d, h, q]
        KT = big.tile([128, H_LOC, S], BF16)   # [d, h, k] (0:64 nope, 64:128 rope)
        V = big.tile([128, ST_N, H_LOC * HD], BF16)  # [s%128, s//128, d_loc]
        ckvT = big.tile([128, LT_N, S], BF16)  # [lat%128, lat//128, s]
        kbuf = big.tile([128, ST_N, H_LOC, HD], BF16)  # [s%128, ST, h, nope||rope]

        # ===== phase 1 scratch =====
        p1 = ExitStack()
        x8p = p1.enter_context(tc.tile_pool(name="x8p", bufs=2))
        scr = p1.enter_context(tc.tile_pool(name="scr", bufs=2))
        jnk = p1.enter_context(tc.tile_pool(name="jnk", bufs=2))

        for kt in range(4):
            nc.sync.dma_start(out=wkd_sbs[0][:, kt], in_=wkd_r[:, kt])
        xq0 = x8p.tile([128, KT_N, X8_CHUNK], BF16, tag="x8")
        xT80 = xT8[0].rearrange("(k p) s -> p k s", p=128)
        for k4 in range(0, KT_N, 4):
            nc.sync.dma_start(out=xq0[:, k4:k4 + 4, :], in_=xT80[:, k4:k4 + 4, :])
        for kt in range(4, KT_N):
            nc.sync.dma_start(out=wkd_sbs[kt // 4][:, kt % 4], in_=wkd_r[:, kt])
        for kt in range(KT_N):
            nc.sync.dma_start(out=wa_sb[:, kt, :], in_=wa_r[:, kt, :])
        nc.sync.dma_start(out=wup_sb[:], in_=w_up[:].rearrange("(k p) n -> p k n", p=128))
        nc.sync.dma_start(out=cos_sb[:], in_=cos4[:].rearrange("(t p) h f -> p t h f", p=128))
        nc.sync.dma_start(out=sin_sb[:], in_=sin4[:].rearrange("(t p) h f -> p t h f", p=128))
        nc.sync.dma_start(out=gain_sb[:], in_=gain12[:])
        nc.sync.dma_start(out=ones_sb[:], in_=ones_in[:])
        nc.sync.dma_start(out=ident_sb[:], in_=ident_in[:])
        nc.vector.memset(eps_sb[:], EPS)

        def rsqrt_act(dst, src, n, bias):
            """dst = 1/sqrt(src/n + eps): ACT Sqrt then fast DVE reciprocal."""
            nc.scalar.activation(dst, src, SQRTF, scale=1.0 / n, bias=bias)
            nc.vector.reciprocal_approx_fast(out=dst, in_=dst)

        def rope_rot(view, cos_ap, sin_ap, out1, out2):
            """out1 = x1*c + x2*s ; out2 = x2*c - x1*s (batched over heads).

            view: [128, H_LOC, 64] (normalized rope input; may alias out1/out2)
            """
            RH = ROPE // 2
            x1 = view[:, :, 0:RH]
            x2 = view[:, :, RH:ROPE]
            t1 = scr.tile([128, H_LOC, RH], F32, tag="t1")
            t2 = scr.tile([128, H_LOC, RH], F32, tag="t2")
            t3 = scr.tile([128, H_LOC, RH], F32, tag="t3")
            t4 = scr.tile([128, H_LOC, RH], F32, tag="t4")
            nc.vector.tensor_tensor(t1[:], x1, cos_ap, MULT)
            nc.vector.tensor_tensor(t2[:], x2, sin_ap, MULT)
            nc.vector.tensor_tensor(t3[:], x2, cos_ap, MULT)
            nc.vector.tensor_tensor(t4[:], x1, sin_ap, MULT)
            nc.vector.tensor_tensor(out1, t1[:], t2[:], ADD)
            nc.vector.tensor_tensor(out2, t3[:], t4[:], SUB)

        # ========== phase 1: kv_down^T + GEMM-A + norms + rope + kv_up =====
        with (
            tc.tile_pool(name="psA", bufs=2, space="PSUM") as psA,
            tc.tile_pool(name="psKD", bufs=1, space="PSUM") as psKD,
            tc.tile_pool(name="psT", bufs=3, space="PSUM") as psT,
            tc.tile_pool(name="psLQ", bufs=1, space="PSUM") as psLQ,
        ):
            def kvd_pass(p2, xq, ec0):
                kdp = psKD.tile([128, 2, 512], F32, tag="KD")
                for kt in range(KT_N):
                    for l2 in range(2):
                        nc.tensor.matmul(
                            kdp[:, l2, 0:X8_CHUNK],
                            wkd_sbs[kt // 4][:, kt % 4, 2 * p2 + l2, :],
                            xq[:, kt, :],
                            start=(kt == 0), stop=(kt == KT_N - 1))
                nc.scalar.copy(
                    ckvT[:, 2 * p2:2 * p2 + 2, ec0:ec0 + X8_CHUNK],
                    kdp[:, :, 0:X8_CHUNK])

            def ckv_norm(ec0):
                """rms-normalize this chunk's raw ckvT in place: squares on
                ACT, lt-fold on DVE, partition reduce via a ones-matmul,
                then gpsimd broadcast of the per-column scale."""
                ckv_e = ckvT[:, :, ec0:ec0 + X8_CHUNK]
                sqkd = jnk.tile([128, LT_N, X8_CHUNK], BF16, tag="sqkd")
                nc.scalar.activation(sqkd[:], ckv_e, SQF)
                ssum = scr.tile([128, X8_CHUNK], BF16, tag="ssum")
                with nc.allow_low_precision(reason="4-way lt fold of bf16 squares"):
                    nc.vector.tensor_reduce(
                        ssum[:], sqkd[:].rearrange("p l s -> p s l"), AXX, ADD)
                lsq = psLQ.tile([1, X8_CHUNK], F32, tag="lsq")
                nc.tensor.matmul(lsq[:], ones_sb[:], ssum[:],
                                 start=True, stop=True)
                rsq = scr.tile([1, X8_CHUNK], F32, tag="rsq")
                nc.scalar.activation(rsq[:], lsq[:], SQRTF,
                                     scale=1.0 / LAT, bias=eps_sb[0:1])
                nc.vector.reciprocal_approx_fast(out=rsq[:], in_=rsq[:])
                rbcs = scr.tile([128, X8_CHUNK], F32, tag="rbcs")
                nc.gpsimd.partition_broadcast(rbcs[:], rsq[:])
                for lt in range(LT_N):
                    nc.vector.tensor_tensor(
                        ckvT[:, lt, ec0:ec0 + X8_CHUNK],
                        ckvT[:, lt, ec0:ec0 + X8_CHUNK], rbcs[:], MULT)

            def gemma_part(ST, xs):
                s0 = ST * 128
                aq = psA.tile([128, 512], F32, tag="A")
                ak = psA.tile([128, 512], F32, tag="A")
                for kt in range(KT_N):
                    nc.tensor.matmul(
                        aq[:], xs[:, kt, :], wa_sb[:, kt, 0:512],
                        start=(kt == 0), stop=(kt == KT_N - 1))
                    nc.tensor.matmul(
                        ak[:, 0:256], xs[:, kt, :], wa_sb[:, kt, 512:768],
                        start=(kt == 0), stop=(kt == KT_N - 1))

                # ---- evict + batched stats ----
                asb = scr.tile([128, A_W], F32, tag="asb")
                nc.scalar.copy(asb[:, 0:512], aq[:])
                nc.scalar.copy(asb[:, 512:768], ak[:, 0:256])
                junk = jnk.tile([128, A_W], BF16, tag="junk")
                nc.scalar.activation(junk[:, 0:512], aq[:], SQF)
                nc.scalar.activation(junk[:, 512:768], ak[:, 0:256], SQF)
                rs12 = scr.tile([128, 12], F32, tag="rs12")
                nc.vector.tensor_reduce(
                    rs12[:],
                    junk[:].rearrange("p (g c) -> p g c", c=64),
                    AXX, ADD)
                rsqrt_act(rs12[:], rs12[:], 64, eps_sb[:])
                nc.vector.tensor_tensor(rs12[:], rs12[:], gain_sb[:], MULT)

                # ---- apply norms (q+krope, 12 groups of 64) ----
                nrm = scr.tile([128, A_W], BF16, tag="nrm")
                nc.vector.tensor_tensor(
                    nrm[:].rearrange("p (g c) -> p g c", c=64),
                    asb[:].rearrange("p (g c) -> p g c", c=64),
                    rs12[:].to_broadcast([128, 12, 64]), MULT)

                # ---- rope (in-place on nrm / into kbuf) ----
                RH = ROPE // 2
                nrm_q = nrm[:, 0:A_QW].rearrange(
                    "p (h t c) -> p h t c", t=2, c=64)
                qro = nrm_q[:, :, 1, :]
                rope_rot(qro, cos_sb[:, ST], sin_sb[:, ST],
                         qro[:, :, 0:RH], qro[:, :, RH:ROPE])
                kro = nrm[:, A_QW:A_W].rearrange("p (h c) -> p h c", c=64)
                rope_rot(kro, cos_sb[:, ST], sin_sb[:, ST],
                         kbuf[:, ST, :, NOPE:NOPE + RH],
                         kbuf[:, ST, :, NOPE + RH:HD])

                return ST, nrm

            def gemma_tr(ST, nrm):
                # q transposes (nope||rope in one shot per head); emitted
                # after later matmuls so the PE never waits on the norm/rope
                # vector chain
                s0 = ST * 128
                for h in range(H_LOC):
                    tq = psT.tile([128, 128], BF16, tag="tq")
                    nc.tensor.transpose(
                        tq[:], nrm[:, h * HD:(h + 1) * HD], ident_sb[:])
                    nc.scalar.copy(QT[:, h, s0:s0 + 128], tq[:])

            def g3_part(ST):
                s0 = ST * 128
                # (shares the psA ring: same shapes, evicted promptly)
                kv1 = psA.tile([128, 512], F32, tag="A")
                kv2 = psA.tile([128, 512], F32, tag="A")
                for lt in range(LT_N):
                    lhs = ckvT[:, lt, s0:s0 + 128]
                    nc.tensor.matmul(
                        kv1[:], lhs, wup_sb[:, lt, 0:512],
                        start=(lt == 0), stop=(lt == LT_N - 1))
                    nc.tensor.matmul(
                        kv2[:, 0:256], lhs, wup_sb[:, lt, 512:768],
                        start=(lt == 0), stop=(lt == LT_N - 1))
                # k_nope batched norm straight into kbuf[:, ST, :, 0:64]
                kvev = scr.tile([128, H_LOC * NOPE], F32, tag="kvev")
                nc.scalar.copy(kvev[:], kv1[:, 0:H_LOC * NOPE])
                junkk = jnk.tile([128, H_LOC * NOPE], BF16, tag="junkk")
                nc.scalar.activation(junkk[:], kv1[:, 0:H_LOC * NOPE], SQF)
                rsk = scr.tile([128, H_LOC], F32, tag="rsk")
                nc.vector.tensor_reduce(
                    rsk[:], junkk[:].rearrange("p (g c) -> p g c", c=64),
                    AXX, ADD)
                rsqrt_act(rsk[:], rsk[:], 64, eps_sb[:])
                nc.vector.tensor_tensor(
                    kbuf[:, ST, :, 0:NOPE],
                    kvev[:].rearrange("p (g c) -> p g c", c=64),
                    rsk[:].to_broadcast([128, H_LOC, 64]), MULT)
                # V evict (split across the two chain tiles)
                nc.scalar.copy(V[:, ST, 0:256], kv1[:, 256:512])
                nc.scalar.copy(V[:, ST, 256:512], kv2[:, 0:256])
                return ST

            def g3_tr(ST):
                # k transposes (nope||rope in one shot per head), deferred
                # past later matmuls to cover the k_nope norm chain
                s0 = ST * 128
                for h in range(H_LOC):
                    tk = psT.tile([128, 128], BF16, tag="tq")
                    nc.tensor.transpose(tk[:], kbuf[:, ST, h, :], ident_sb[:])
                    nc.vector.tensor_copy(KT[:, h, s0:s0 + 128], tk[:])

            _p1sc = nc.enter_named_scope("p1", False)[0]
            pend_kt = None
            for e in range(X8_N):
                if e == 0:
                    xq = xq0
                else:
                    xq = x8p.tile([128, KT_N, X8_CHUNK], BF16, tag="x8")
                    nc.sync.dma_start(
                        out=xq[:], in_=xT8[e].rearrange("(k p) s -> p k s", p=128))
                ec0 = e * X8_CHUNK
                # every vector chain (norm/rope, latent norm, k_nope norm) is
                # covered by the next block of independent matmuls before any
                # transpose that consumes it is issued
                kvd_pass(0, xq, ec0)
                if pend_kt is not None:
                    g3_tr(pend_kt)
                gm0 = gemma_part(2 * e, xq[:, :, 0:128])
                kvd_pass(1, xq, ec0)
                gemma_tr(*gm0)
                ckv_norm(ec0)
                gm1 = gemma_part(2 * e + 1, xq[:, :, 128:256])
                g30 = g3_part(2 * e)
                gemma_tr(*gm1)
                g31 = g3_part(2 * e + 1)
                g3_tr(g30)
                pend_kt = g31
            g3_tr(pend_kt)
            nc.leave_named_scope("p1", _p1sc, False)

        p1.close()

        if debug_taps:
            nc.sync.dma_start(out=dbg_ckvT[:], in_=ckvT[:])
            nc.sync.dma_start(out=dbg_QT[:], in_=QT[:])
            nc.sync.dma_start(out=dbg_KT[:], in_=KT[:])
            nc.sync.dma_start(out=dbg_V[:], in_=V[:])

        # late const loads (attention/proj only)
        nc.sync.dma_start(out=mask_sb[:], in_=masks[:])
        nc.sync.dma_start(out=wp_sb[:], in_=w_p[:].rearrange("(k p) n -> p k n", p=128))

        # ====== phase 3: attention + out projection (fused per q-block) ======
        # Software-pipelined k-loop over PAIRS of 128-k-tiles: one exp per
        # pair (halves ACT instruction+access overhead), scores of pair n+1
        # emitted before V-matmuls of pair n so the PE never waits on exp.
        # Softmax denominator: P accumulated pairwise on DVE, reduced over
        # partitions by gpsimd (no PE ones-matmuls, no PSUM bank).
        inv_sqrt_hd = 1.0 / math.sqrt(HD)
        with (
            tc.tile_pool(name="pP", bufs=6) as pP,
            tc.tile_pool(name="pAcc", bufs=3) as pAcc,
            tc.tile_pool(name="pL", bufs=2) as pL,
            tc.tile_pool(name="pRb", bufs=2) as pRb,
            tc.tile_pool(name="pY", bufs=2) as pY,
            tc.tile_pool(name="pO", bufs=4) as pO,
            tc.tile_pool(name="psS", bufs=2, space="PSUM") as psS,
            tc.tile_pool(name="psY", bufs=2, space="PSUM") as psY,
            tc.tile_pool(name="psL", bufs=2, space="PSUM") as psL,
        ):
            def emit_proj(j, yT):
                # accumulators borrow the score-pair ring (never open at the
                # same time as score chains; only lane 0's bank is used)
                q0 = j * QB
                for sub in range(QB // 128):
                    sq0 = q0 + sub * 128
                    for nb in range(D // NB):
                        opst = psS.tile([128, 2, QB], F32, tag="S")
                        ops = opst[:, 0, :]
                        for h in range(H_LOC):
                            nc.tensor.matmul(
                                ops, yT[:, h, sub * 128:(sub + 1) * 128],
                                wp_sb[:, h, nb * NB:(nb + 1) * NB],
                                start=(h == 0), stop=(h == H_LOC - 1))
                        osb = pO.tile([128, NB], BF16, tag="osb")
                        if nb % 4 == 0:
                            nc.scalar.copy(osb[:], ops)
                        else:
                            nc.vector.tensor_copy(osb[:], ops)
                        nc.sync.dma_start(
                            out=out[sq0:sq0 + 128, nb * NB:(nb + 1) * NB],
                            in_=osb[:])

            _p3sc = nc.enter_named_scope("p3", False)[0]
            prev_proj = None
            for j in range(S // QB):
                q0 = j * QB
                nkt = (q0 + QB) // 128
                npair = nkt // 2
                dpair0 = (q0 // 128) // 2  # first diagonal pair index
                yT = pY.tile([128, H_LOC, QB], BF16, tag="yT")

                def fin_head(h, yT, yps, pacc):
                    # softmax denominator + normalize, deferred into the next
                    # head's k-loop so the PE never waits on the pacc chain
                    lps = psL.tile([1, QB], F32, tag="L")
                    for i in range(2):
                        nc.tensor.matmul(
                            lps[:], ones_sb[:], pacc[:, i, :],
                            start=(i == 0), stop=(i == 1))
                    r = pL.tile([1, QB], F32, tag="r")
                    nc.vector.reciprocal_approx_fast(out=r[:], in_=lps[:])
                    rbc = pRb.tile([128, QB], F32, tag="rbc")
                    nc.gpsimd.partition_broadcast(rbc[:], r[:])
                    nc.vector.tensor_tensor(
                        yT[:, h, :], yps[:], rbc[:], MULT)

                pfin = None
                for h in range(H_LOC):
                    yps = psY.tile([128, QB], F32, tag="Y")
                    pacc = pAcc.tile([128, 2, QB], BF16, tag="acc")
                    prev = None
                    for kp in range(npair + 1):
                        if kp == 1 and pfin is not None:
                            fin_head(*pfin)
                            pfin = None
                        if kp < npair:
                            spair = psS.tile([128, 2, QB], F32, tag="S")
                            for i in range(2):
                                kt = 2 * kp + i
                                nc.tensor.matmul(
                                    spair[:, i, :],
                                    KT[:, h, kt * 128:(kt + 1) * 128],
                                    QT[:, h, q0:q0 + QB],
                                    start=True, stop=True)
                            Pp = pP.tile([128, 2, QB], BF16, tag="P")
                            nc.scalar.activation(
                                Pp[:], spair[:], EXPF, scale=inv_sqrt_hd)
                            if kp >= dpair0:
                                di = 2 * (kp - dpair0)
                                nc.vector.tensor_tensor(
                                    Pp[:], Pp[:], mask_sb[:, di:di + 2, :],
                                    MULT)
                            if kp == 0:
                                nc.vector.tensor_copy(pacc[:], Pp[:])
                            else:
                                nc.vector.tensor_tensor(
                                    pacc[:], pacc[:], Pp[:], ADD)
                        if prev is not None:
                            pkp, pP_t = prev
                            for i in range(2):
                                kt = 2 * pkp + i
                                nc.tensor.matmul(
                                    yps[:], V[:, kt, h * HD:(h + 1) * HD],
                                    pP_t[:, i, :],
                                    start=(kt == 0), stop=(kt == nkt - 1))
                        if kp < npair:
                            prev = (kp, Pp)
                    pfin = (h, yT, yps, pacc)

                # ---- deferred projection: emit previous j's proj here so
                # its PE work fills this j's ACT-bound attention gaps; the
                # last head's finalize chain flows underneath it ----
                if prev_proj is not None:
                    emit_proj(*prev_proj)
                fin_head(*pfin)
                prev_proj = (j, yT)
            emit_proj(*prev_proj)
            nc.leave_named_scope("p3", _p3sc, False)
    nc.compile()
    return nc


def _prep_inputs(x, w_q_krope, w_kv_down, w_kv_up, w_proj, q_gain):
    """Build the 8 per-core input maps (host-side sharding)."""
    inv_freq = ROPE_BASE ** (-np.arange(0, ROPE, 2, dtype=np.float32) / ROPE)
    t = np.arange(S, dtype=np.float32)
    freqs = np.outer(t, inv_freq)                      # (S, 32)
    cos4 = np.ascontiguousarray(np.broadcast_to(
        np.cos(freqs)[:, None, :], (S, H_LOC, ROPE // 2))).astype(BF)
    sin4 = np.ascontiguousarray(np.broadcast_to(
        np.sin(freqs)[:, None, :], (S, H_LOC, ROPE // 2))).astype(BF)

    kk = np.arange(128)[:, None, None]
    dd = np.arange(4)[None, :, None]
    qq = np.arange(QB)[None, None, :]
    masks = (kk + 128 * dd <= qq).astype(BF)           # [128, 4, QB]

    ones_in = np.ones((128, 1), dtype=BF)
    ident_in = np.eye(128, dtype=np.float32).astype(BF)

    # x^T per batch, chunked: [X8_N, D, X8_CHUNK]
    xT_chunks = []
    for b in range(B):
        xT = np.ascontiguousarray(x[b].T).astype(BF)   # [D, S]
        xT_chunks.append(np.ascontiguousarray(
            xT.reshape(D, X8_N, X8_CHUNK).transpose(1, 0, 2)))

    w_kd = np.ascontiguousarray(w_kv_down).astype(BF)  # [D, LAT]

    in_maps = []
    for c in range(N_CORES):
        b = c // H_LOC
        hg = c % H_LOC
        heads = [hg * H_LOC + i for i in range(H_LOC)]
        w_a = np.concatenate(
            [w_q_krope[:, h * HD:(h + 1) * HD] for h in heads]
            + [w_q_krope[:, D + h * ROPE:D + (h + 1) * ROPE] for h in heads],
            axis=1).astype(BF)                          # [D, 768]
        w_up = np.concatenate(
            [w_kv_up[:, h * NOPE:(h + 1) * NOPE] for h in heads]
            + [w_kv_up[:, NOPE * H + h * HD:NOPE * H + (h + 1) * HD]
               for h in heads], axis=1).astype(BF)      # [LAT, 768]
        w_p = w_proj[hg * DLOC:(hg + 1) * DLOC, :].astype(BF)   # [512, D]
        g = q_gain[heads].astype(np.float32)
        g12 = np.concatenate([np.repeat(g, 2), np.ones(4, np.float32)])
        gain12 = np.ascontiguousarray(
            np.broadcast_to(g12[None, :], (128, 12))).astype(np.float32)
        # this core's group-rank quarter of x^T for the latent path
        xkv = np.ascontiguousarray(xT_chunks[b][2 * hg:2 * hg + 2])
        in_maps.append({
            "xT8": xT_chunks[b],
            "xkv": xkv,
            "w_a": np.ascontiguousarray(w_a),
            "w_kd": w_kd,
            "w_up": np.ascontiguousarray(w_up),
            "w_p": np.ascontiguousarray(w_p),
            "cos4": cos4, "sin4": sin4, "masks": masks,
            "gain12": gain12,
            "ones_in": ones_in, "ident_in": ident_in,
        })
    return in_maps


def kernel(x, w_q_krope, w_kv_down, w_kv_up, w_proj, q_gain, **_unused):
    x = np.asarray(x, dtype=np.float32)
    w_q_krope = np.asarray(w_q_krope, dtype=np.float32)
    w_kv_down = np.asarray(w_kv_down, dtype=np.float32)
    w_kv_up = np.asarray(w_kv_up, dtype=np.float32)
    w_proj = np.asarray(w_proj, dtype=np.float32)
    q_gain = np.asarray(q_gain, dtype=np.float32)

    if "nc" not in _PROGRAM_CACHE:
        _PROGRAM_CACHE["nc"] = _build_program()
    nc = _PROGRAM_CACHE["nc"]

    in_maps = _prep_inputs(x, w_q_krope, w_kv_down, w_kv_up, w_proj, q_gain)
    res = run_bass_kernel_spmd(nc, in_maps, list(range(N_CORES)))

    out = np.zeros((B, S, D), dtype=np.float32)
    for c in range(N_CORES):
        out[c // H_LOC] += res.results[c]["out"]
    return out


# revision 51
# speedup vs baseline: 1.0099x; 1.0099x over previous
"""Multi-head latent attention (MLA) forward pass on 8 Trainium2 NeuronCores.

Sharding: 2 (batch) x 4 (head-group) grid. Core c handles batch b = c // 4
and heads 4*(c % 4) .. 4*(c % 4) + 3.  Per core:
  - streams x[b]^T (host-pretransposed, bf16) once
  - kv_down GEMM in TRANSPOSED form: ckv^T[lat, s] = Wkd_chunk^T.T @ x^T,
    rms-norm over lat via ones-matmul (partition reduce) + gpsimd broadcast;
    born-transposed => no PE transposes for the latent.
  - GEMM-A: A[s, 768] = x_s @ [Wq_heads | Wkrope_heads]; batched rms-norms;
    rope applied in-place; ONE 128-wide PE transpose per head for q
    (nope||rope together) into QT.
  - GEMM-3 (kv_up) fused per s-tile (uses just-computed ckv^T columns);
    k_nope normed into kbuf[0:64], k_rope (phase A) in kbuf[64:128] =>
    ONE 128-wide transpose per head for k into KT.
  - causal attention per head in transposed-score form; softmax denominator
    via DVE-accumulated P (groups of 4 k-tiles) + one ones-matmul per group
    instead of one per k-tile.
  - out projection fused per q-block j; partials DMA'd as computed.
Host sums the 4 partials per batch element.
"""

import sys

for _p in ("/opt/trn_rl_repo",):
    if _p not in sys.path:
        sys.path.insert(0, _p)

import math
from contextlib import ExitStack

import ml_dtypes
import numpy as np

import concourse.bass as bass
import concourse.bass_isa as bass_isa
import concourse.mybir as mybir
import concourse.tile as tile
from concourse import bacc
from concourse.bass_utils import run_bass_kernel_spmd

F32 = mybir.dt.float32
BF16 = mybir.dt.bfloat16
BF = ml_dtypes.bfloat16

B, S, D = 2, 2048, 2048
H = 16
HD = 128           # head dim
ROPE = 64
NOPE = 64
LAT = 512
EPS = 1e-6
ROPE_BASE = 10000.0

H_LOC = 4          # heads per core
N_CORES = 8
DLOC = H_LOC * HD  # 512, per-core proj contraction size

ST_N = S // 128    # 16 s-tiles
KT_N = D // 128    # 16 k-tiles for GEMM-A
QB = 512           # attention q-block width
NB = 512           # proj output block width
LGRP = 4           # k-tiles per softmax-denominator accumulation group

A_QW = H_LOC * HD             # 512  q columns in A
A_RW = H_LOC * ROPE           # 256  k_rope columns in A
A_W = A_QW + A_RW             # 768 total A columns (kv_down separate)
KV_W = H_LOC * NOPE + H_LOC * HD   # 768 kv columns
LT_N = LAT // 128  # 4 latent tiles

X8_CHUNK = 256                # s-columns of x^T per streamed chunk
X8_N = S // X8_CHUNK          # 8 chunks
ST_PER_CHUNK = X8_CHUNK // 128  # 2

MULT = mybir.AluOpType.mult
ADD = mybir.AluOpType.add
SUB = mybir.AluOpType.subtract
EXPF = mybir.ActivationFunctionType.Exp
SQRTF = mybir.ActivationFunctionType.Sqrt
SQF = mybir.ActivationFunctionType.Square
AXX = mybir.AxisListType.X

_PROGRAM_CACHE = {}


def _build_program(debug_taps=False):
    nc = bacc.Bacc(None, target_bir_lowering=False, debug=True)

    # ---- DRAM I/O ----
    xT8 = nc.dram_tensor("xT8", [X8_N, D, X8_CHUNK], BF16, kind="ExternalInput")
    xkv = nc.dram_tensor("xkv", [2, D, X8_CHUNK], BF16, kind="ExternalInput")
    w_a = nc.dram_tensor("w_a", [D, A_W], BF16, kind="ExternalInput")
    w_kd = nc.dram_tensor("w_kd", [D, LAT], BF16, kind="ExternalInput")
    w_up = nc.dram_tensor("w_up", [LAT, KV_W], BF16, kind="ExternalInput")
    w_p = nc.dram_tensor("w_p", [DLOC, D], BF16, kind="ExternalInput")
    cos4 = nc.dram_tensor("cos4", [S, H_LOC, ROPE // 2], BF16, kind="ExternalInput")
    sin4 = nc.dram_tensor("sin4", [S, H_LOC, ROPE // 2], BF16, kind="ExternalInput")
    masks = nc.dram_tensor("masks", [128, 4, QB], BF16, kind="ExternalInput")
    gain12 = nc.dram_tensor("gain12", [128, 12], F32, kind="ExternalInput")
    ones_in = nc.dram_tensor("ones_in", [128, 1], BF16, kind="ExternalInput")
    ident_in = nc.dram_tensor("ident_in", [128, 128], BF16, kind="ExternalInput")
    out = nc.dram_tensor("out", [S, D], BF16, kind="ExternalOutput")
    if debug_taps:
        dbg_ckvT = nc.dram_tensor("dbg_ckvT", [128, LT_N, S], BF16, kind="ExternalOutput")
        dbg_QT = nc.dram_tensor("dbg_QT", [128, H_LOC, S], BF16, kind="ExternalOutput")
        dbg_KT = nc.dram_tensor("dbg_KT", [128, H_LOC, S], BF16, kind="ExternalOutput")
        dbg_V = nc.dram_tensor("dbg_V", [128, ST_N, H_LOC * HD], BF16, kind="ExternalOutput")

    with tile.TileContext(nc) as tc, ExitStack() as top:
        const = top.enter_context(tc.tile_pool(name="const", bufs=1))
        big = top.enter_context(tc.tile_pool(name="big", bufs=1))

        # --- resident weights/constants, load order = consumption order ---
        wkd_sbs = [const.tile([128, 4, LT_N, 128], BF16, name=f"wkd{i}")
                   for i in range(4)]
        wkd_r = w_kd[:].rearrange("(k p) (l q) -> p k l q", p=128, q=128)
        wa_sb = const.tile([128, KT_N, A_W], BF16)
        wa_r = w_a[:].rearrange("(k p) n -> p k n", p=128)
        wup_sb = const.tile([128, LT_N, KV_W], BF16)
        cos_sb = const.tile([128, ST_N, H_LOC, ROPE // 2], BF16)
        sin_sb = const.tile([128, ST_N, H_LOC, ROPE // 2], BF16)
        gain_sb = const.tile([128, 12], F32)
        ones_sb = const.tile([128, 1], BF16)
        ident_sb = const.tile([128, 128], BF16)
        eps_sb = const.tile([128, 1], F32)
        # loaded late (only needed by attention/proj phase)
        mask_sb = const.tile([128, 4, QB], BF16)
        wp_sb = const.tile([128, H_LOC, D], BF16)

        # --- persistent activations (head-dim-major) ---
        QT = big.tile([128, H_LOC, S], BF16)   # [d, h, q]
        KT = big.tile([128, H_LOC, S], BF16)   # [d, h, k] (0:64 nope, 64:128 rope)
        V = big.tile([128, ST_N, H_LOC * HD], BF16)  # [s%128, s//128, d_loc]
        ckvT = big.tile([128, LT_N, S], BF16)  # [lat%128, lat//128, s]
        kbuf = big.tile([128, ST_N, H_LOC, HD], BF16)  # [s%128, ST, h, nope||rope]

        # ===== phase 1 scratch =====
        p1 = ExitStack()
        x8p = p1.enter_context(tc.tile_pool(name="x8p", bufs=2))
        scr = p1.enter_context(tc.tile_pool(name="scr", bufs=2))
        jnk = p1.enter_context(tc.tile_pool(name="jnk", bufs=2))

        for kt in range(4):
            nc.sync.dma_start(out=wkd_sbs[0][:, kt], in_=wkd_r[:, kt])
        xq0 = x8p.tile([128, KT_N, X8_CHUNK], BF16, tag="x8")
        xT80 = xT8[0].rearrange("(k p) s -> p k s", p=128)
        for k4 in range(0, KT_N, 4):
            nc.sync.dma_start(out=xq0[:, k4:k4 + 4, :], in_=xT80[:, k4:k4 + 4, :])
        for kt in range(4, KT_N):
            nc.sync.dma_start(out=wkd_sbs[kt // 4][:, kt % 4], in_=wkd_r[:, kt])
        for kt in range(KT_N):
            nc.sync.dma_start(out=wa_sb[:, kt, :], in_=wa_r[:, kt, :])
        nc.sync.dma_start(out=wup_sb[:], in_=w_up[:].rearrange("(k p) n -> p k n", p=128))
        nc.sync.dma_start(out=cos_sb[:], in_=cos4[:].rearrange("(t p) h f -> p t h f", p=128))
        nc.sync.dma_start(out=sin_sb[:], in_=sin4[:].rearrange("(t p) h f -> p t h f", p=128))
        nc.sync.dma_start(out=gain_sb[:], in_=gain12[:])
        nc.sync.dma_start(out=ones_sb[:], in_=ones_in[:])
        nc.sync.dma_start(out=ident_sb[:], in_=ident_in[:])
        nc.vector.memset(eps_sb[:], EPS)

        def rsqrt_act(dst, src, n, bias):
            """dst = 1/sqrt(src/n + eps): ACT Sqrt then fast DVE reciprocal."""
            nc.scalar.activation(dst, src, SQRTF, scale=1.0 / n, bias=bias)
            nc.vector.reciprocal_approx_fast(out=dst, in_=dst)

        def rope_rot(view, cos_ap, sin_ap, out1, out2):
            """out1 = x1*c + x2*s ; out2 = x2*c - x1*s (batched over heads).

            view: [128, H_LOC, 64] (normalized rope input; may alias out1/out2)
            """
            RH = ROPE // 2
            x1 = view[:, :, 0:RH]
            x2 = view[:, :, RH:ROPE]
            t1 = scr.tile([128, H_LOC, RH], F32, tag="t1")
            t2 = scr.tile([128, H_LOC, RH], F32, tag="t2")
            t3 = scr.tile([128, H_LOC, RH], F32, tag="t3")
            t4 = scr.tile([128, H_LOC, RH], F32, tag="t4")
            nc.vector.tensor_tensor(t1[:], x1, cos_ap, MULT)
            nc.vector.tensor_tensor(t2[:], x2, sin_ap, MULT)
            nc.vector.tensor_tensor(t3[:], x2, cos_ap, MULT)
            nc.vector.tensor_tensor(t4[:], x1, sin_ap, MULT)
            nc.vector.tensor_tensor(out1, t1[:], t2[:], ADD)
            nc.vector.tensor_tensor(out2, t3[:], t4[:], SUB)

        # ========== phase 1: kv_down^T + GEMM-A + norms + rope + kv_up =====
        with (
            tc.tile_pool(name="psA", bufs=2, space="PSUM") as psA,
            tc.tile_pool(name="psKD", bufs=1, space="PSUM") as psKD,
            tc.tile_pool(name="psT", bufs=3, space="PSUM") as psT,
            tc.tile_pool(name="psLQ", bufs=1, space="PSUM") as psLQ,
        ):
            def kvd_pass(p2, xq, ec0):
                kdp = psKD.tile([128, 2, 512], F32, tag="KD")
                for kt in range(KT_N):
                    for l2 in range(2):
                        nc.tensor.matmul(
                            kdp[:, l2, 0:X8_CHUNK],
                            wkd_sbs[kt // 4][:, kt % 4, 2 * p2 + l2, :],
                            xq[:, kt, :],
                            start=(kt == 0), stop=(kt == KT_N - 1))
                nc.scalar.copy(
                    ckvT[:, 2 * p2:2 * p2 + 2, ec0:ec0 + X8_CHUNK],
                    kdp[:, :, 0:X8_CHUNK])

            def ckv_norm(ec0):
                """rms-normalize this chunk's raw ckvT in place: squares on
                ACT, lt-fold on DVE, partition reduce via a ones-matmul,
                then gpsimd broadcast of the per-column scale."""
                ckv_e = ckvT[:, :, ec0:ec0 + X8_CHUNK]
                sqkd = jnk.tile([128, LT_N, X8_CHUNK], BF16, tag="sqkd")
                nc.scalar.activation(sqkd[:], ckv_e, SQF)
                ssum = scr.tile([128, X8_CHUNK], BF16, tag="ssum")
                with nc.allow_low_precision(reason="4-way lt fold of bf16 squares"):
                    nc.vector.tensor_reduce(
                        ssum[:], sqkd[:].rearrange("p l s -> p s l"), AXX, ADD)
                lsq = psLQ.tile([1, X8_CHUNK], F32, tag="lsq")
                nc.tensor.matmul(lsq[:], ones_sb[:], ssum[:],
                                 start=True, stop=True)
                rsq = scr.tile([1, X8_CHUNK], F32, tag="rsq")
                nc.scalar.activation(rsq[:], lsq[:], SQRTF,
                                     scale=1.0 / LAT, bias=eps_sb[0:1])
                nc.vector.reciprocal_approx_fast(out=rsq[:], in_=rsq[:])
                rbcs = scr.tile([128, X8_CHUNK], F32, tag="rbcs")
                nc.gpsimd.partition_broadcast(rbcs[:], rsq[:])
                for lt in range(LT_N):
                    nc.vector.tensor_tensor(
                        ckvT[:, lt, ec0:ec0 + X8_CHUNK],
                        ckvT[:, lt, ec0:ec0 + X8_CHUNK], rbcs[:], MULT)

            def gemma_part(ST, xs):
                s0 = ST * 128
                aq = psA.tile([128, 512], F32, tag="A")
                ak = psA.tile([128, 512], F32, tag="A")
                for kt in range(KT_N):
                    nc.tensor.matmul(
                        aq[:], xs[:, kt, :], wa_sb[:, kt, 0:512],
                        start=(kt == 0), stop=(kt == KT_N - 1))
                    nc.tensor.matmul(
                        ak[:, 0:256], xs[:, kt, :], wa_sb[:, kt, 512:768],
                        start=(kt == 0), stop=(kt == KT_N - 1))

                # ---- evict + batched stats ----
                asb = scr.tile([128, A_W], F32, tag="asb")
                nc.scalar.copy(asb[:, 0:512], aq[:])
                nc.scalar.copy(asb[:, 512:768], ak[:, 0:256])
                junk = jnk.tile([128, A_W], BF16, tag="junk")
                nc.scalar.activation(junk[:, 0:512], aq[:], SQF)
                nc.scalar.activation(junk[:, 512:768], ak[:, 0:256], SQF)
                rs12 = scr.tile([128, 12], F32, tag="rs12")
                nc.vector.tensor_reduce(
                    rs12[:],
                    junk[:].rearrange("p (g c) -> p g c", c=64),
                    AXX, ADD)
                rsqrt_act(rs12[:], rs12[:], 64, eps_sb[:])
                nc.vector.tensor_tensor(rs12[:], rs12[:], gain_sb[:], MULT)

                # ---- apply norms (q+krope, 12 groups of 64) ----
                nrm = scr.tile([128, A_W], BF16, tag="nrm")
                nc.vector.tensor_tensor(
                    nrm[:].rearrange("p (g c) -> p g c", c=64),
                    asb[:].rearrange("p (g c) -> p g c", c=64),
                    rs12[:].to_broadcast([128, 12, 64]), MULT)

                # ---- rope (in-place on nrm / into kbuf) ----
                RH = ROPE // 2
                nrm_q = nrm[:, 0:A_QW].rearrange(
                    "p (h t c) -> p h t c", t=2, c=64)
                qro = nrm_q[:, :, 1, :]
                rope_rot(qro, cos_sb[:, ST], sin_sb[:, ST],
                         qro[:, :, 0:RH], qro[:, :, RH:ROPE])
                kro = nrm[:, A_QW:A_W].rearrange("p (h c) -> p h c", c=64)
                rope_rot(kro, cos_sb[:, ST], sin_sb[:, ST],
                         kbuf[:, ST, :, NOPE:NOPE + RH],
                         kbuf[:, ST, :, NOPE + RH:HD])

                return ST, nrm

            def gemma_tr(ST, nrm):
                # q transposes (nope||rope in one shot per head); emitted
                # after later matmuls so the PE never waits on the norm/rope
                # vector chain
                s0 = ST * 128
                for h in range(H_LOC):
                    tq = psT.tile([128, 128], BF16, tag="tq")
                    nc.tensor.transpose(
                        tq[:], nrm[:, h * HD:(h + 1) * HD], ident_sb[:])
                    nc.scalar.copy(QT[:, h, s0:s0 + 128], tq[:])

            def g3_part(ST):
                s0 = ST * 128
                # (shares the psA ring: same shapes, evicted promptly)
                kv1 = psA.tile([128, 512], F32, tag="A")
                kv2 = psA.tile([128, 512], F32, tag="A")
                for lt in range(LT_N):
                    lhs = ckvT[:, lt, s0:s0 + 128]
                    nc.tensor.matmul(
                        kv1[:], lhs, wup_sb[:, lt, 0:512],
                        start=(lt == 0), stop=(lt == LT_N - 1))
                    nc.tensor.matmul(
                        kv2[:, 0:256], lhs, wup_sb[:, lt, 512:768],
                        start=(lt == 0), stop=(lt == LT_N - 1))
                # k_nope batched norm straight into kbuf[:, ST, :, 0:64]
                kvev = scr.tile([128, H_LOC * NOPE], F32, tag="kvev")
                nc.scalar.copy(kvev[:], kv1[:, 0:H_LOC * NOPE])
                junkk = jnk.tile([128, H_LOC * NOPE], BF16, tag="junkk")
                nc.scalar.activation(junkk[:], kv1[:, 0:H_LOC * NOPE], SQF)
                rsk = scr.tile([128, H_LOC], F32, tag="rsk")
                nc.vector.tensor_reduce(
                    rsk[:], junkk[:].rearrange("p (g c) -> p g c", c=64),
                    AXX, ADD)
                rsqrt_act(rsk[:], rsk[:], 64, eps_sb[:])
                nc.vector.tensor_tensor(
                    kbuf[:, ST, :, 0:NOPE],
                    kvev[:].rearrange("p (g c) -> p g c", c=64),
                    rsk[:].to_broadcast([128, H_LOC, 64]), MULT)
                # V evict (split across the two chain tiles)
                nc.scalar.copy(V[:, ST, 0:256], kv1[:, 256:512])
                nc.scalar.copy(V[:, ST, 256:512], kv2[:, 0:256])
                return ST

            def g3_tr(ST):
                # k transposes (nope||rope in one shot per head), deferred
                # past later matmuls to cover the k_nope norm chain
                s0 = ST * 128
                for h in range(H_LOC):
                    tk = psT.tile([128, 128], BF16, tag="tq")
                    nc.tensor.transpose(tk[:], kbuf[:, ST, h, :], ident_sb[:])
                    nc.vector.tensor_copy(KT[:, h, s0:s0 + 128], tk[:])

            _p1sc = nc.enter_named_scope("p1", False)[0]
            for e in range(X8_N):
                if e == 0:
                    xq = xq0
                else:
                    xq = x8p.tile([128, KT_N, X8_CHUNK], BF16, tag="x8")
                    nc.sync.dma_start(
                        out=xq[:], in_=xT8[e].rearrange("(k p) s -> p k s", p=128))
                ec0 = e * X8_CHUNK
                # every vector chain (norm/rope, latent norm, k_nope norm) is
                # covered by the next block of independent matmuls before any
                # transpose that consumes it is issued
                kvd_pass(0, xq, ec0)
                gm0 = gemma_part(2 * e, xq[:, :, 0:128])
                kvd_pass(1, xq, ec0)
                gemma_tr(*gm0)
                ckv_norm(ec0)
                gm1 = gemma_part(2 * e + 1, xq[:, :, 128:256])
                g30 = g3_part(2 * e)
                gemma_tr(*gm1)
                g31 = g3_part(2 * e + 1)
                g3_tr(g30)
                g3_tr(g31)
            nc.leave_named_scope("p1", _p1sc, False)

        p1.close()

        if debug_taps:
            nc.sync.dma_start(out=dbg_ckvT[:], in_=ckvT[:])
            nc.sync.dma_start(out=dbg_QT[:], in_=QT[:])
            nc.sync.dma_start(out=dbg_KT[:], in_=KT[:])
            nc.sync.dma_start(out=dbg_V[:], in_=V[:])

        # late const loads (attention/proj only)
        nc.sync.dma_start(out=mask_sb[:], in_=masks[:])
        nc.sync.dma_start(out=wp_sb[:], in_=w_p[:].rearrange("(k p) n -> p k n", p=128))

        # ====== phase 3: attention + out projection (fused per q-block) ======
        # Software-pipelined k-loop over PAIRS of 128-k-tiles: one exp per
        # pair (halves ACT instruction+access overhead), scores of pair n+1
        # emitted before V-matmuls of pair n so the PE never waits on exp.
        # Softmax denominator: P accumulated pairwise on DVE, reduced over
        # partitions by gpsimd (no PE ones-matmuls, no PSUM bank).
        inv_sqrt_hd = 1.0 / math.sqrt(HD)
        with (
            tc.tile_pool(name="pP", bufs=6) as pP,
            tc.tile_pool(name="pAcc", bufs=3) as pAcc,
            tc.tile_pool(name="pL", bufs=2) as pL,
            tc.tile_pool(name="pRb", bufs=2) as pRb,
            tc.tile_pool(name="pY", bufs=2) as pY,
            tc.tile_pool(name="pO", bufs=4) as pO,
            tc.tile_pool(name="psS", bufs=2, space="PSUM") as psS,
            tc.tile_pool(name="psY", bufs=2, space="PSUM") as psY,
            tc.tile_pool(name="psL", bufs=2, space="PSUM") as psL,
        ):
            def emit_proj(j, yT):
                # accumulators borrow the score-pair ring (never open at the
                # same time as score chains; only lane 0's bank is used)
                q0 = j * QB
                for sub in range(QB // 128):
                    sq0 = q0 + sub * 128
                    for nb in range(D // NB):
                        opst = psS.tile([128, 2, QB], F32, tag="S")
                        ops = opst[:, 0, :]
                        for h in range(H_LOC):
                            nc.tensor.matmul(
                                ops, yT[:, h, sub * 128:(sub + 1) * 128],
                                wp_sb[:, h, nb * NB:(nb + 1) * NB],
                                start=(h == 0), stop=(h == H_LOC - 1))
                        osb = pO.tile([128, NB], BF16, tag="osb")
                        if nb % 4 == 0:
                            nc.scalar.copy(osb[:], ops)
                        else:
                            nc.vector.tensor_copy(osb[:], ops)
                        nc.sync.dma_start(
                            out=out[sq0:sq0 + 128, nb * NB:(nb + 1) * NB],
                            in_=osb[:])

            _p3sc = nc.enter_named_scope("p3", False)[0]
            prev_proj = None
            for j in range(S // QB):
                q0 = j * QB
                nkt = (q0 + QB) // 128
                npair = nkt // 2
                dpair0 = (q0 // 128) // 2  # first diagonal pair index
                yT = pY.tile([128, H_LOC, QB], BF16, tag="yT")

                def fin_head(h, yT, yps, pacc):
                    # softmax denominator + normalize, deferred into the next
                    # head's k-loop so the PE never waits on the pacc chain
                    lps = psL.tile([1, QB], F32, tag="L")
                    for i in range(2):
                        nc.tensor.matmul(
                            lps[:], ones_sb[:], pacc[:, i, :],
                            start=(i == 0), stop=(i == 1))
                    r = pL.tile([1, QB], F32, tag="r")
                    nc.vector.reciprocal_approx_fast(out=r[:], in_=lps[:])
                    rbc = pRb.tile([128, QB], F32, tag="rbc")
                    nc.gpsimd.partition_broadcast(rbc[:], r[:])
                    nc.vector.tensor_tensor(
                        yT[:, h, :], yps[:], rbc[:], MULT)

                pfin = None
                for h in range(H_LOC):
                    yps = psY.tile([128, QB], F32, tag="Y")
                    pacc = pAcc.tile([128, 2, QB], BF16, tag="acc")
                    prev = None
                    for kp in range(npair + 1):
                        if kp == 1 and pfin is not None:
                            fin_head(*pfin)
                            pfin = None
                        if kp < npair:
                            spair = psS.tile([128, 2, QB], F32, tag="S")
                            for i in range(2):
                                kt = 2 * kp + i
                                nc.tensor.matmul(
                                    spair[:, i, :],
                                    KT[:, h, kt * 128:(kt + 1) * 128],
                                    QT[:, h, q0:q0 + QB],
                                    start=True, stop=True)
                            Pp = pP.tile([128, 2, QB], BF16, tag="P")
                            nc.scalar.activation(
                                Pp[:], spair[:], EXPF, scale=inv_sqrt_hd)
                            if kp >= dpair0:
                                di = 2 * (kp - dpair0)
                                nc.vector.tensor_tensor(
                                    Pp[:], Pp[:], mask_sb[:, di:di + 2, :],
                                    MULT)
                            if kp == 0:
                                nc.vector.tensor_copy(pacc[:], Pp[:])
                            else:
                                nc.vector.tensor_tensor(
                                    pacc[:], pacc[:], Pp[:], ADD)
                        if prev is not None:
                            pkp, pP_t = prev
                            for i in range(2):
                                kt = 2 * pkp + i
                                nc.tensor.matmul(
                                    yps[:], V[:, kt, h * HD:(h + 1) * HD],
                                    pP_t[:, i, :],
                                    start=(kt == 0), stop=(kt == nkt - 1))
                        if kp < npair:
                            prev = (kp, Pp)
                    pfin = (h, yT, yps, pacc)

                # ---- deferred projection: emit previous j's proj here so
                # its PE work fills this j's ACT-bound attention gaps; the
                # last head's finalize chain flows underneath it ----
                if prev_proj is not None:
                    emit_proj(*prev_proj)
                fin_head(*pfin)
                prev_proj = (j, yT)
            emit_proj(*prev_proj)
            nc.leave_named_scope("p3", _p3sc, False)
    nc.compile()
    return nc


def _prep_inputs(x, w_q_krope, w_kv_down, w_kv_up, w_proj, q_gain):
    """Build the 8 per-core input maps (host-side sharding)."""
    inv_freq = ROPE_BASE ** (-np.arange(0, ROPE, 2, dtype=np.float32) / ROPE)
    t = np.arange(S, dtype=np.float32)
    freqs = np.outer(t, inv_freq)                      # (S, 32)
    cos4 = np.ascontiguousarray(np.broadcast_to(
        np.cos(freqs)[:, None, :], (S, H_LOC, ROPE // 2))).astype(BF)
    sin4 = np.ascontiguousarray(np.broadcast_to(
        np.sin(freqs)[:, None, :], (S, H_LOC, ROPE // 2))).astype(BF)

    kk = np.arange(128)[:, None, None]
    dd = np.arange(4)[None, :, None]
    qq = np.arange(QB)[None, None, :]
    masks = (kk + 128 * dd <= qq).astype(BF)           # [128, 4, QB]

    ones_in = np.ones((128, 1), dtype=BF)
    ident_in = np.eye(128, dtype=np.float32).astype(BF)

    # x^T per batch, chunked: [X8_N, D, X8_CHUNK]
    xT_chunks = []
    for b in range(B):
        xT = np.ascontiguousarray(x[b].T).astype(BF)   # [D, S]
        xT_chunks.append(np.ascontiguousarray(
            xT.reshape(D, X8_N, X8_CHUNK).transpose(1, 0, 2)))

    w_kd = np.ascontiguousarray(w_kv_down).astype(BF)  # [D, LAT]

    in_maps = []
    for c in range(N_CORES):
        b = c // H_LOC
        hg = c % H_LOC
        heads = [hg * H_LOC + i for i in range(H_LOC)]
        w_a = np.concatenate(
            [w_q_krope[:, h * HD:(h + 1) * HD] for h in heads]
            + [w_q_krope[:, D + h * ROPE:D + (h + 1) * ROPE] for h in heads],
            axis=1).astype(BF)                          # [D, 768]
        w_up = np.concatenate(
            [w_kv_up[:, h * NOPE:(h + 1) * NOPE] for h in heads]
            + [w_kv_up[:, NOPE * H + h * HD:NOPE * H + (h + 1) * HD]
               for h in heads], axis=1).astype(BF)      # [LAT, 768]
        w_p = w_proj[hg * DLOC:(hg + 1) * DLOC, :].astype(BF)   # [512, D]
        g = q_gain[heads].astype(np.float32)
        g12 = np.concatenate([np.repeat(g, 2), np.ones(4, np.float32)])
        gain12 = np.ascontiguousarray(
            np.broadcast_to(g12[None, :], (128, 12))).astype(np.float32)
        # this core's group-rank quarter of x^T for the latent path
        xkv = np.ascontiguousarray(xT_chunks[b][2 * hg:2 * hg + 2])
        in_maps.append({
            "xT8": xT_chunks[b],
            "xkv": xkv,
            "w_a": np.ascontiguousarray(w_a),
            "w_kd": w_kd,
            "w_up": np.ascontiguousarray(w_up),
            "w_p": np.ascontiguousarray(w_p),
            "cos4": cos4, "sin4": sin4, "masks": masks,
            "gain12": gain12,
            "ones_in": ones_in, "ident_in": ident_in,
        })
    return in_maps


def kernel(x, w_q_krope, w_kv_down, w_kv_up, w_proj, q_gain, **_unused):
    x = np.asarray(x, dtype=np.float32)
    w_q_krope = np.asarray(w_q_krope, dtype=np.float32)
    w_kv_down = np.asarray(w_kv_down, dtype=np.float32)
    w_kv_up = np.asarray(w_kv_up, dtype=np.float32)
    w_proj = np.asarray(w_proj, dtype=np.float32)
    q_gain = np.asarray(q_gain, dtype=np.float32)

    if "nc" not in _PROGRAM_CACHE:
        _PROGRAM_CACHE["nc"] = _build_program()
    nc = _PROGRAM_CACHE["nc"]

    in_maps = _prep_inputs(x, w_q_krope, w_kv_down, w_kv_up, w_proj, q_gain)
    res = run_bass_kernel_spmd(nc, in_maps, list(range(N_CORES)))

    out = np.zeros((B, S, D), dtype=np.float32)
    for c in range(N_CORES):
        out[c // H_LOC] += res.results[c]["out"]
    return out


# revision 53
# speedup vs baseline: 1.0113x; 1.0014x over previous
"""Multi-head latent attention (MLA) forward pass on 8 Trainium2 NeuronCores.

Sharding: 2 (batch) x 4 (head-group) grid. Core c handles batch b = c // 4
and heads 4*(c % 4) .. 4*(c % 4) + 3.  Per core:
  - streams x[b]^T (host-pretransposed, bf16) once
  - kv_down GEMM in TRANSPOSED form: ckv^T[lat, s] = Wkd_chunk^T.T @ x^T,
    rms-norm over lat via ones-matmul (partition reduce) + gpsimd broadcast;
    born-transposed => no PE transposes for the latent.
  - GEMM-A: A[s, 768] = x_s @ [Wq_heads | Wkrope_heads]; batched rms-norms;
    rope applied in-place; ONE 128-wide PE transpose per head for q
    (nope||rope together) into QT.
  - GEMM-3 (kv_up) fused per s-tile (uses just-computed ckv^T columns);
    k_nope normed into kbuf[0:64], k_rope (phase A) in kbuf[64:128] =>
    ONE 128-wide transpose per head for k into KT.
  - causal attention per head in transposed-score form; softmax denominator
    via DVE-accumulated P (groups of 4 k-tiles) + one ones-matmul per group
    instead of one per k-tile.
  - out projection fused per q-block j; partials DMA'd as computed.
Host sums the 4 partials per batch element.
"""

import sys

for _p in ("/opt/trn_rl_repo",):
    if _p not in sys.path:
        sys.path.insert(0, _p)

import math
from contextlib import ExitStack

import ml_dtypes
import numpy as np

import concourse.bass as bass
import concourse.bass_isa as bass_isa
import concourse.mybir as mybir
import concourse.tile as tile
from concourse import bacc
from concourse.bass_utils import run_bass_kernel_spmd

F32 = mybir.dt.float32
BF16 = mybir.dt.bfloat16
BF = ml_dtypes.bfloat16

B, S, D = 2, 2048, 2048
H = 16
HD = 128           # head dim
ROPE = 64
NOPE = 64
LAT = 512
EPS = 1e-6
ROPE_BASE = 10000.0

H_LOC = 4          # heads per core
N_CORES = 8
DLOC = H_LOC * HD  # 512, per-core proj contraction size

ST_N = S // 128    # 16 s-tiles
KT_N = D // 128    # 16 k-tiles for GEMM-A
QB = 512           # attention q-block width
NB = 512           # proj output block width
LGRP = 4           # k-tiles per softmax-denominator accumulation group

A_QW = H_LOC * HD             # 512  q columns in A
A_RW = H_LOC * ROPE           # 256  k_rope columns in A
A_W = A_QW + A_RW             # 768 total A columns (kv_down separate)
KV_W = H_LOC * NOPE + H_LOC * HD   # 768 kv columns
LT_N = LAT // 128  # 4 latent tiles

X8_CHUNK = 256                # s-columns of x^T per streamed chunk
X8_N = S // X8_CHUNK          # 8 chunks
ST_PER_CHUNK = X8_CHUNK // 128  # 2

MULT = mybir.AluOpType.mult
ADD = mybir.AluOpType.add
SUB = mybir.AluOpType.subtract
EXPF = mybir.ActivationFunctionType.Exp
SQRTF = mybir.ActivationFunctionType.Sqrt
SQF = mybir.ActivationFunctionType.Square
AXX = mybir.AxisListType.X

_PROGRAM_CACHE = {}


def _build_program(debug_taps=False):
    nc = bacc.Bacc(None, target_bir_lowering=False, debug=True)

    # ---- DRAM I/O ----
    xT8 = nc.dram_tensor("xT8", [X8_N, D, X8_CHUNK], BF16, kind="ExternalInput")
    xkv = nc.dram_tensor("xkv", [2, D, X8_CHUNK], BF16, kind="ExternalInput")
    w_a = nc.dram_tensor("w_a", [D, A_W], BF16, kind="ExternalInput")
    w_kd = nc.dram_tensor("w_kd", [D, LAT], BF16, kind="ExternalInput")
    w_up = nc.dram_tensor("w_up", [LAT, KV_W], BF16, kind="ExternalInput")
    w_p = nc.dram_tensor("w_p", [DLOC, D], BF16, kind="ExternalInput")
    cos4 = nc.dram_tensor("cos4", [S, H_LOC, ROPE // 2], BF16, kind="ExternalInput")
    sin4 = nc.dram_tensor("sin4", [S, H_LOC, ROPE // 2], BF16, kind="ExternalInput")
    masks = nc.dram_tensor("masks", [128, 4, QB], BF16, kind="ExternalInput")
    gain12 = nc.dram_tensor("gain12", [128, 12], F32, kind="ExternalInput")
    ones_in = nc.dram_tensor("ones_in", [128, 1], BF16, kind="ExternalInput")
    ident_in = nc.dram_tensor("ident_in", [128, 128], BF16, kind="ExternalInput")
    out = nc.dram_tensor("out", [S, D], BF16, kind="ExternalOutput")
    if debug_taps:
        dbg_ckvT = nc.dram_tensor("dbg_ckvT", [128, LT_N, S], BF16, kind="ExternalOutput")
        dbg_QT = nc.dram_tensor("dbg_QT", [128, H_LOC, S], BF16, kind="ExternalOutput")
        dbg_KT = nc.dram_tensor("dbg_KT", [128, H_LOC, S], BF16, kind="ExternalOutput")
        dbg_V = nc.dram_tensor("dbg_V", [128, ST_N, H_LOC * HD], BF16, kind="ExternalOutput")

    with tile.TileContext(nc) as tc, ExitStack() as top:
        const = top.enter_context(tc.tile_pool(name="const", bufs=1))
        big = top.enter_context(tc.tile_pool(name="big", bufs=1))

        # --- resident weights/constants, load order = consumption order ---
        wkd_sbs = [const.tile([128, 4, LT_N, 128], BF16, name=f"wkd{i}")
                   for i in range(4)]
        wkd_r = w_kd[:].rearrange("(k p) (l q) -> p k l q", p=128, q=128)
        wa_sb = const.tile([128, KT_N, A_W], BF16)
        wa_r = w_a[:].rearrange("(k p) n -> p k n", p=128)
        wup_sb = const.tile([128, LT_N, KV_W], BF16)
        cos_sb = const.tile([128, ST_N, H_LOC, ROPE // 2], BF16)
        sin_sb = const.tile([128, ST_N, H_LOC, ROPE // 2], BF16)
        gain_sb = const.tile([128, 12], F32)
        ones_sb = const.tile([128, 1], BF16)
        ident_sb = const.tile([128, 128], BF16)
        eps_sb = const.tile([128, 1], F32)
        # loaded late (only needed by attention/proj phase)
        mask_sb = const.tile([128, 4, QB], BF16)
        wp_sb = const.tile([128, H_LOC, D], BF16)

        # --- persistent activations (head-dim-major) ---
        QT = big.tile([128, H_LOC, S], BF16)   # [d, h, q]
        KT = big.tile([128, H_LOC, S], BF16)   # [d, h, k] (0:64 nope, 64:128 rope)
        V = big.tile([128, ST_N, H_LOC * HD], BF16)  # [s%128, s//128, d_loc]
        ckvT = big.tile([128, LT_N, S], BF16)  # [lat%128, lat//128, s]
        kbuf = big.tile([128, ST_N, H_LOC, HD], BF16)  # [s%128, ST, h, nope||rope]

        # ===== phase 1 scratch =====
        p1 = ExitStack()
        x8p = p1.enter_context(tc.tile_pool(name="x8p", bufs=2))
        scr = p1.enter_context(tc.tile_pool(name="scr", bufs=2))
        jnk = p1.enter_context(tc.tile_pool(name="jnk", bufs=2))

        for kt in range(4):
            nc.sync.dma_start(out=wkd_sbs[0][:, kt], in_=wkd_r[:, kt])
        xq0 = x8p.tile([128, KT_N, X8_CHUNK], BF16, tag="x8")
        xT80 = xT8[0].rearrange("(k p) s -> p k s", p=128)
        for k4 in range(0, KT_N, 4):
            nc.sync.dma_start(out=xq0[:, k4:k4 + 4, :], in_=xT80[:, k4:k4 + 4, :])
        for kt in range(4, KT_N):
            nc.sync.dma_start(out=wkd_sbs[kt // 4][:, kt % 4], in_=wkd_r[:, kt])
        for kt in range(KT_N):
            nc.sync.dma_start(out=wa_sb[:, kt, :], in_=wa_r[:, kt, :])
        nc.sync.dma_start(out=wup_sb[:], in_=w_up[:].rearrange("(k p) n -> p k n", p=128))
        nc.sync.dma_start(out=cos_sb[:], in_=cos4[:].rearrange("(t p) h f -> p t h f", p=128))
        nc.sync.dma_start(out=sin_sb[:], in_=sin4[:].rearrange("(t p) h f -> p t h f", p=128))
        nc.sync.dma_start(out=gain_sb[:], in_=gain12[:])
        nc.sync.dma_start(out=ones_sb[:], in_=ones_in[:])
        nc.sync.dma_start(out=ident_sb[:], in_=ident_in[:])
        nc.vector.memset(eps_sb[:], EPS)

        def rsqrt_act(dst, src, n, bias):
            """dst = 1/sqrt(src/n + eps): ACT Sqrt then fast DVE reciprocal."""
            nc.scalar.activation(dst, src, SQRTF, scale=1.0 / n, bias=bias)
            nc.vector.reciprocal_approx_fast(out=dst, in_=dst)

        def rope_rot(view, cos_ap, sin_ap, out1, out2):
            """out1 = x1*c + x2*s ; out2 = x2*c - x1*s (batched over heads).

            view: [128, H_LOC, 64] (normalized rope input; may alias out1/out2)
            """
            RH = ROPE // 2
            x1 = view[:, :, 0:RH]
            x2 = view[:, :, RH:ROPE]
            t1 = scr.tile([128, H_LOC, RH], F32, tag="t1")
            t2 = scr.tile([128, H_LOC, RH], F32, tag="t2")
            t3 = scr.tile([128, H_LOC, RH], F32, tag="t3")
            t4 = scr.tile([128, H_LOC, RH], F32, tag="t4")
            nc.vector.tensor_tensor(t1[:], x1, cos_ap, MULT)
            nc.vector.tensor_tensor(t2[:], x2, sin_ap, MULT)
            nc.vector.tensor_tensor(t3[:], x2, cos_ap, MULT)
            nc.vector.tensor_tensor(t4[:], x1, sin_ap, MULT)
            nc.vector.tensor_tensor(out1, t1[:], t2[:], ADD)
            nc.vector.tensor_tensor(out2, t3[:], t4[:], SUB)

        # ========== phase 1: kv_down^T + GEMM-A + norms + rope + kv_up =====
        with (
            tc.tile_pool(name="psA", bufs=2, space="PSUM") as psA,
            tc.tile_pool(name="psKD", bufs=1, space="PSUM") as psKD,
            tc.tile_pool(name="psT", bufs=3, space="PSUM") as psT,
            tc.tile_pool(name="psLQ", bufs=1, space="PSUM") as psLQ,
        ):
            def kvd_pass(p2, xq, ec0):
                kdp = psKD.tile([128, 2, 512], F32, tag="KD")
                for kt in range(KT_N):
                    for l2 in range(2):
                        nc.tensor.matmul(
                            kdp[:, l2, 0:X8_CHUNK],
                            wkd_sbs[kt // 4][:, kt % 4, 2 * p2 + l2, :],
                            xq[:, kt, :],
                            start=(kt == 0), stop=(kt == KT_N - 1))
                nc.scalar.copy(
                    ckvT[:, 2 * p2:2 * p2 + 2, ec0:ec0 + X8_CHUNK],
                    kdp[:, :, 0:X8_CHUNK])

            def ckv_norm(ec0):
                """rms-normalize this chunk's raw ckvT in place: squares on
                ACT, lt-fold on DVE, partition reduce via a ones-matmul,
                then gpsimd broadcast of the per-column scale."""
                ckv_e = ckvT[:, :, ec0:ec0 + X8_CHUNK]
                sqkd = jnk.tile([128, LT_N, X8_CHUNK], BF16, tag="sqkd")
                nc.scalar.activation(sqkd[:], ckv_e, SQF)
                ssum = scr.tile([128, X8_CHUNK], BF16, tag="ssum")
                with nc.allow_low_precision(reason="4-way lt fold of bf16 squares"):
                    nc.vector.tensor_reduce(
                        ssum[:], sqkd[:].rearrange("p l s -> p s l"), AXX, ADD)
                lsq = psLQ.tile([1, X8_CHUNK], F32, tag="lsq")
                nc.tensor.matmul(lsq[:], ones_sb[:], ssum[:],
                                 start=True, stop=True)
                rsq = scr.tile([1, X8_CHUNK], F32, tag="rsq")
                nc.scalar.activation(rsq[:], lsq[:], SQRTF,
                                     scale=1.0 / LAT, bias=eps_sb[0:1])
                nc.vector.reciprocal_approx_fast(out=rsq[:], in_=rsq[:])
                rbcs = scr.tile([128, X8_CHUNK], F32, tag="rbcs")
                nc.gpsimd.partition_broadcast(rbcs[:], rsq[:])
                for lt in range(LT_N):
                    nc.vector.tensor_tensor(
                        ckvT[:, lt, ec0:ec0 + X8_CHUNK],
                        ckvT[:, lt, ec0:ec0 + X8_CHUNK], rbcs[:], MULT)

            def gemma_part(ST, xs):
                s0 = ST * 128
                aq = psA.tile([128, 512], F32, tag="A")
                ak = psA.tile([128, 512], F32, tag="A")
                for kt in range(KT_N):
                    nc.tensor.matmul(
                        aq[:], xs[:, kt, :], wa_sb[:, kt, 0:512],
                        start=(kt == 0), stop=(kt == KT_N - 1))
                    nc.tensor.matmul(
                        ak[:, 0:256], xs[:, kt, :], wa_sb[:, kt, 512:768],
                        start=(kt == 0), stop=(kt == KT_N - 1))

                # ---- evict + batched stats ----
                asb = scr.tile([128, A_W], F32, tag="asb")
                nc.scalar.copy(asb[:, 0:512], aq[:])
                nc.scalar.copy(asb[:, 512:768], ak[:, 0:256])
                junk = jnk.tile([128, A_W], BF16, tag="junk")
                nc.scalar.activation(junk[:, 0:512], aq[:], SQF)
                nc.scalar.activation(junk[:, 512:768], ak[:, 0:256], SQF)
                rs12 = scr.tile([128, 12], F32, tag="rs12")
                nc.vector.tensor_reduce(
                    rs12[:],
                    junk[:].rearrange("p (g c) -> p g c", c=64),
                    AXX, ADD)
                rsqrt_act(rs12[:], rs12[:], 64, eps_sb[:])
                nc.vector.tensor_tensor(rs12[:], rs12[:], gain_sb[:], MULT)

                # ---- apply norms (q+krope, 12 groups of 64) ----
                nrm = scr.tile([128, A_W], BF16, tag="nrm")
                nc.vector.tensor_tensor(
                    nrm[:].rearrange("p (g c) -> p g c", c=64),
                    asb[:].rearrange("p (g c) -> p g c", c=64),
                    rs12[:].to_broadcast([128, 12, 64]), MULT)

                # ---- rope (in-place on nrm / into kbuf) ----
                RH = ROPE // 2
                nrm_q = nrm[:, 0:A_QW].rearrange(
                    "p (h t c) -> p h t c", t=2, c=64)
                qro = nrm_q[:, :, 1, :]
                rope_rot(qro, cos_sb[:, ST], sin_sb[:, ST],
                         qro[:, :, 0:RH], qro[:, :, RH:ROPE])
                kro = nrm[:, A_QW:A_W].rearrange("p (h c) -> p h c", c=64)
                rope_rot(kro, cos_sb[:, ST], sin_sb[:, ST],
                         kbuf[:, ST, :, NOPE:NOPE + RH],
                         kbuf[:, ST, :, NOPE + RH:HD])

                return ST, nrm

            def gemma_tr(ST, nrm):
                # q transposes (nope||rope in one shot per head); emitted
                # after later matmuls so the PE never waits on the norm/rope
                # vector chain
                s0 = ST * 128
                for h in range(H_LOC):
                    tq = psT.tile([128, 128], BF16, tag="tq")
                    nc.tensor.transpose(
                        tq[:], nrm[:, h * HD:(h + 1) * HD], ident_sb[:])
                    nc.scalar.copy(QT[:, h, s0:s0 + 128], tq[:])

            def g3_part(ST):
                s0 = ST * 128
                # (shares the psA ring: same shapes, evicted promptly)
                kv1 = psA.tile([128, 512], F32, tag="A")
                kv2 = psA.tile([128, 512], F32, tag="A")
                for lt in range(LT_N):
                    lhs = ckvT[:, lt, s0:s0 + 128]
                    nc.tensor.matmul(
                        kv1[:], lhs, wup_sb[:, lt, 0:512],
                        start=(lt == 0), stop=(lt == LT_N - 1))
                    nc.tensor.matmul(
                        kv2[:, 0:256], lhs, wup_sb[:, lt, 512:768],
                        start=(lt == 0), stop=(lt == LT_N - 1))
                # k_nope batched norm straight into kbuf[:, ST, :, 0:64]
                kvev = scr.tile([128, H_LOC * NOPE], F32, tag="kvev")
                nc.scalar.copy(kvev[:], kv1[:, 0:H_LOC * NOPE])
                junkk = jnk.tile([128, H_LOC * NOPE], BF16, tag="junkk")
                nc.scalar.activation(junkk[:], kv1[:, 0:H_LOC * NOPE], SQF)
                rsk = scr.tile([128, H_LOC], F32, tag="rsk")
                nc.vector.tensor_reduce(
                    rsk[:], junkk[:].rearrange("p (g c) -> p g c", c=64),
                    AXX, ADD)
                rsqrt_act(rsk[:], rsk[:], 64, eps_sb[:])
                nc.vector.tensor_tensor(
                    kbuf[:, ST, :, 0:NOPE],
                    kvev[:].rearrange("p (g c) -> p g c", c=64),
                    rsk[:].to_broadcast([128, H_LOC, 64]), MULT)
                # V evict (split across the two chain tiles)
                nc.scalar.copy(V[:, ST, 0:256], kv1[:, 256:512])
                nc.scalar.copy(V[:, ST, 256:512], kv2[:, 0:256])
                return ST

            def g3_tr(ST):
                # k transposes (nope||rope in one shot per head), deferred
                # past later matmuls to cover the k_nope norm chain
                s0 = ST * 128
                for h in range(H_LOC):
                    tk = psT.tile([128, 128], BF16, tag="tq")
                    nc.tensor.transpose(tk[:], kbuf[:, ST, h, :], ident_sb[:])
                    nc.vector.tensor_copy(KT[:, h, s0:s0 + 128], tk[:])

            _p1sc = nc.enter_named_scope("p1", False)[0]
            for e in range(X8_N):
                if e == 0:
                    xq = xq0
                else:
                    xq = x8p.tile([128, KT_N, X8_CHUNK], BF16, tag="x8")
                    nc.sync.dma_start(
                        out=xq[:], in_=xT8[e].rearrange("(k p) s -> p k s", p=128))
                ec0 = e * X8_CHUNK
                # every vector chain (norm/rope, latent norm, k_nope norm) is
                # covered by the next block of independent matmuls before any
                # transpose that consumes it is issued
                kvd_pass(0, xq, ec0)
                gm0 = gemma_part(2 * e, xq[:, :, 0:128])
                kvd_pass(1, xq, ec0)
                gemma_tr(*gm0)
                ckv_norm(ec0)
                gm1 = gemma_part(2 * e + 1, xq[:, :, 128:256])
                g30 = g3_part(2 * e)
                gemma_tr(*gm1)
                g31 = g3_part(2 * e + 1)
                g3_tr(g30)
                g3_tr(g31)
            nc.leave_named_scope("p1", _p1sc, False)

        p1.close()

        if debug_taps:
            nc.sync.dma_start(out=dbg_ckvT[:], in_=ckvT[:])
            nc.sync.dma_start(out=dbg_QT[:], in_=QT[:])
            nc.sync.dma_start(out=dbg_KT[:], in_=KT[:])
            nc.sync.dma_start(out=dbg_V[:], in_=V[:])

        # late const loads (attention/proj only)
        nc.sync.dma_start(out=mask_sb[:], in_=masks[:])
        nc.sync.dma_start(out=wp_sb[:], in_=w_p[:].rearrange("(k p) n -> p k n", p=128))

        # ====== phase 3: attention + out projection (fused per q-block) ======
        # Software-pipelined k-loop over PAIRS of 128-k-tiles: one exp per
        # pair (halves ACT instruction+access overhead), scores of pair n+1
        # emitted before V-matmuls of pair n so the PE never waits on exp.
        # Softmax denominator: P accumulated pairwise on DVE, reduced over
        # partitions by gpsimd (no PE ones-matmuls, no PSUM bank).
        inv_sqrt_hd = 1.0 / math.sqrt(HD)
        with (
            tc.tile_pool(name="pP", bufs=6) as pP,
            tc.tile_pool(name="pAcc", bufs=3) as pAcc,
            tc.tile_pool(name="pL", bufs=2) as pL,
            tc.tile_pool(name="pRb", bufs=2) as pRb,
            tc.tile_pool(name="pY", bufs=2) as pY,
            tc.tile_pool(name="pO", bufs=4) as pO,
            tc.tile_pool(name="psS", bufs=2, space="PSUM") as psS,
            tc.tile_pool(name="psY", bufs=2, space="PSUM") as psY,
            tc.tile_pool(name="psL", bufs=2, space="PSUM") as psL,
        ):
            def emit_proj_sub(j, yT, sub):
                # accumulators borrow the score-pair ring (never open at the
                # same time as score chains; only lane 0's bank is used)
                q0 = j * QB
                if True:
                    sq0 = q0 + sub * 128
                    for nb in range(D // NB):
                        opst = psS.tile([128, 2, QB], F32, tag="S")
                        ops = opst[:, 0, :]
                        for h in range(H_LOC):
                            nc.tensor.matmul(
                                ops, yT[:, h, sub * 128:(sub + 1) * 128],
                                wp_sb[:, h, nb * NB:(nb + 1) * NB],
                                start=(h == 0), stop=(h == H_LOC - 1))
                        osb = pO.tile([128, NB], BF16, tag="osb")
                        if nb % 4 == 0:
                            nc.scalar.copy(osb[:], ops)
                        else:
                            nc.vector.tensor_copy(osb[:], ops)
                        nc.sync.dma_start(
                            out=out[sq0:sq0 + 128, nb * NB:(nb + 1) * NB],
                            in_=osb[:])

            _p3sc = nc.enter_named_scope("p3", False)[0]
            prev_proj = None
            for j in range(S // QB):
                q0 = j * QB
                nkt = (q0 + QB) // 128
                npair = nkt // 2
                dpair0 = (q0 // 128) // 2  # first diagonal pair index
                yT = pY.tile([128, H_LOC, QB], BF16, tag="yT")

                def fin_head(h, yT, yps, pacc):
                    # softmax denominator + normalize, deferred into the next
                    # head's k-loop so the PE never waits on the pacc chain
                    lps = psL.tile([1, QB], F32, tag="L")
                    for i in range(2):
                        nc.tensor.matmul(
                            lps[:], ones_sb[:], pacc[:, i, :],
                            start=(i == 0), stop=(i == 1))
                    r = pL.tile([1, QB], F32, tag="r")
                    nc.vector.reciprocal_approx_fast(out=r[:], in_=lps[:])
                    rbc = pRb.tile([128, QB], F32, tag="rbc")
                    nc.gpsimd.partition_broadcast(rbc[:], r[:])
                    nc.vector.tensor_tensor(
                        yT[:, h, :], yps[:], rbc[:], MULT)

                pfin = None
                for h in range(H_LOC):
                    yps = psY.tile([128, QB], F32, tag="Y")
                    pacc = pAcc.tile([128, 2, QB], BF16, tag="acc")
                    prev = None
                    for kp in range(npair + 1):
                        if kp == 1 and pfin is not None:
                            fin_head(*pfin)
                            pfin = None
                        if kp < npair:
                            spair = psS.tile([128, 2, QB], F32, tag="S")
                            for i in range(2):
                                kt = 2 * kp + i
                                nc.tensor.matmul(
                                    spair[:, i, :],
                                    KT[:, h, kt * 128:(kt + 1) * 128],
                                    QT[:, h, q0:q0 + QB],
                                    start=True, stop=True)
                            Pp = pP.tile([128, 2, QB], BF16, tag="P")
                            nc.scalar.activation(
                                Pp[:], spair[:], EXPF, scale=inv_sqrt_hd)
                            if kp >= dpair0:
                                di = 2 * (kp - dpair0)
                                nc.vector.tensor_tensor(
                                    Pp[:], Pp[:], mask_sb[:, di:di + 2, :],
                                    MULT)
                            if kp == 0:
                                nc.vector.tensor_copy(pacc[:], Pp[:])
                            else:
                                nc.vector.tensor_tensor(
                                    pacc[:], pacc[:], Pp[:], ADD)
                        if prev is not None:
                            pkp, pP_t = prev
                            for i in range(2):
                                kt = 2 * pkp + i
                                nc.tensor.matmul(
                                    yps[:], V[:, kt, h * HD:(h + 1) * HD],
                                    pP_t[:, i, :],
                                    start=(kt == 0), stop=(kt == nkt - 1))
                        if kp < npair:
                            prev = (kp, Pp)
                    pfin = (h, yT, yps, pacc)
                    # deferred projection: one quarter of the previous j's
                    # proj after each head, spreading its PE work across
                    # this j's ACT-bound attention
                    if prev_proj is not None:
                        emit_proj_sub(prev_proj[0], prev_proj[1], h)

                fin_head(*pfin)
                prev_proj = (j, yT)
            for sub in range(QB // 128):
                emit_proj_sub(prev_proj[0], prev_proj[1], sub)
            nc.leave_named_scope("p3", _p3sc, False)
    nc.compile()
    return nc


def _prep_inputs(x, w_q_krope, w_kv_down, w_kv_up, w_proj, q_gain):
    """Build the 8 per-core input maps (host-side sharding)."""
    inv_freq = ROPE_BASE ** (-np.arange(0, ROPE, 2, dtype=np.float32) / ROPE)
    t = np.arange(S, dtype=np.float32)
    freqs = np.outer(t, inv_freq)                      # (S, 32)
    cos4 = np.ascontiguousarray(np.broadcast_to(
        np.cos(freqs)[:, None, :], (S, H_LOC, ROPE // 2))).astype(BF)
    sin4 = np.ascontiguousarray(np.broadcast_to(
        np.sin(freqs)[:, None, :], (S, H_LOC, ROPE // 2))).astype(BF)

    kk = np.arange(128)[:, None, None]
    dd = np.arange(4)[None, :, None]
    qq = np.arange(QB)[None, None, :]
    masks = (kk + 128 * dd <= qq).astype(BF)           # [128, 4, QB]

    ones_in = np.ones((128, 1), dtype=BF)
    ident_in = np.eye(128, dtype=np.float32).astype(BF)

    # x^T per batch, chunked: [X8_N, D, X8_CHUNK]
    xT_chunks = []
    for b in range(B):
        xT = np.ascontiguousarray(x[b].T).astype(BF)   # [D, S]
        xT_chunks.append(np.ascontiguousarray(
            xT.reshape(D, X8_N, X8_CHUNK).transpose(1, 0, 2)))

    w_kd = np.ascontiguousarray(w_kv_down).astype(BF)  # [D, LAT]

    in_maps = []
    for c in range(N_CORES):
        b = c // H_LOC
        hg = c % H_LOC
        heads = [hg * H_LOC + i for i in range(H_LOC)]
        w_a = np.concatenate(
            [w_q_krope[:, h * HD:(h + 1) * HD] for h in heads]
            + [w_q_krope[:, D + h * ROPE:D + (h + 1) * ROPE] for h in heads],
            axis=1).astype(BF)                          # [D, 768]
        w_up = np.concatenate(
            [w_kv_up[:, h * NOPE:(h + 1) * NOPE] for h in heads]
            + [w_kv_up[:, NOPE * H + h * HD:NOPE * H + (h + 1) * HD]
               for h in heads], axis=1).astype(BF)      # [LAT, 768]
        w_p = w_proj[hg * DLOC:(hg + 1) * DLOC, :].astype(BF)   # [512, D]
        g = q_gain[heads].astype(np.float32)
        g12 = np.concatenate([np.repeat(g, 2), np.ones(4, np.float32)])
        gain12 = np.ascontiguousarray(
            np.broadcast_to(g12[None, :], (128, 12))).astype(np.float32)
        # this core's group-rank quarter of x^T for the latent path
        xkv = np.ascontiguousarray(xT_chunks[b][2 * hg:2 * hg + 2])
        in_maps.append({
            "xT8": xT_chunks[b],
            "xkv": xkv,
            "w_a": np.ascontiguousarray(w_a),
            "w_kd": w_kd,
            "w_up": np.ascontiguousarray(w_up),
            "w_p": np.ascontiguousarray(w_p),
            "cos4": cos4, "sin4": sin4, "masks": masks,
            "gain12": gain12,
            "ones_in": ones_in, "ident_in": ident_in,
        })
    return in_maps


def kernel(x, w_q_krope, w_kv_down, w_kv_up, w_proj, q_gain, **_unused):
    x = np.asarray(x, dtype=np.float32)
    w_q_krope = np.asarray(w_q_krope, dtype=np.float32)
    w_kv_down = np.asarray(w_kv_down, dtype=np.float32)
    w_kv_up = np.asarray(w_kv_up, dtype=np.float32)
    w_proj = np.asarray(w_proj, dtype=np.float32)
    q_gain = np.asarray(q_gain, dtype=np.float32)

    if "nc" not in _PROGRAM_CACHE:
        _PROGRAM_CACHE["nc"] = _build_program()
    nc = _PROGRAM_CACHE["nc"]

    in_maps = _prep_inputs(x, w_q_krope, w_kv_down, w_kv_up, w_proj, q_gain)
    res = run_bass_kernel_spmd(nc, in_maps, list(range(N_CORES)))

    out = np.zeros((B, S, D), dtype=np.float32)
    for c in range(N_CORES):
        out[c // H_LOC] += res.results[c]["out"]
    return out


# revision 54
# speedup vs baseline: 1.0187x; 1.0073x over previous
"""Multi-head latent attention (MLA) forward pass on 8 Trainium2 NeuronCores.

Sharding: 2 (batch) x 4 (head-group) grid. Core c handles batch b = c // 4
and heads 4*(c % 4) .. 4*(c % 4) + 3.  Per core:
  - streams x[b]^T (host-pretransposed, bf16) once
  - kv_down GEMM in TRANSPOSED form: ckv^T[lat, s] = Wkd_chunk^T.T @ x^T,
    rms-norm over lat via ones-matmul (partition reduce) + gpsimd broadcast;
    born-transposed => no PE transposes for the latent.
  - GEMM-A: A[s, 768] = x_s @ [Wq_heads | Wkrope_heads]; batched rms-norms;
    rope applied in-place; ONE 128-wide PE transpose per head for q
    (nope||rope together) into QT.
  - GEMM-3 (kv_up) fused per s-tile (uses just-computed ckv^T columns);
    k_nope normed into kbuf[0:64], k_rope (phase A) in kbuf[64:128] =>
    ONE 128-wide transpose per head for k into KT.
  - causal attention per head in transposed-score form; softmax denominator
    via DVE-accumulated P (groups of 4 k-tiles) + one ones-matmul per group
    instead of one per k-tile.
  - out projection fused per q-block j; partials DMA'd as computed.
Host sums the 4 partials per batch element.
"""

import sys

for _p in ("/opt/trn_rl_repo",):
    if _p not in sys.path:
        sys.path.insert(0, _p)

import math
from contextlib import ExitStack

import ml_dtypes
import numpy as np

import concourse.bass as bass
import concourse.bass_isa as bass_isa
import concourse.mybir as mybir
import concourse.tile as tile
from concourse import bacc
from concourse.bass_utils import run_bass_kernel_spmd

F32 = mybir.dt.float32
BF16 = mybir.dt.bfloat16
BF = ml_dtypes.bfloat16

B, S, D = 2, 2048, 2048
H = 16
HD = 128           # head dim
ROPE = 64
NOPE = 64
LAT = 512
EPS = 1e-6
ROPE_BASE = 10000.0

H_LOC = 4          # heads per core
N_CORES = 8
DLOC = H_LOC * HD  # 512, per-core proj contraction size

ST_N = S // 128    # 16 s-tiles
KT_N = D // 128    # 16 k-tiles for GEMM-A
QB = 512           # attention q-block width
NB = 512           # proj output block width
LGRP = 4           # k-tiles per softmax-denominator accumulation group

A_QW = H_LOC * HD             # 512  q columns in A
A_RW = H_LOC * ROPE           # 256  k_rope columns in A
A_W = A_QW + A_RW             # 768 total A columns (kv_down separate)
KV_W = H_LOC * NOPE + H_LOC * HD   # 768 kv columns
LT_N = LAT // 128  # 4 latent tiles

X8_CHUNK = 256                # s-columns of x^T per streamed chunk
X8_N = S // X8_CHUNK          # 8 chunks
ST_PER_CHUNK = X8_CHUNK // 128  # 2

MULT = mybir.AluOpType.mult
ADD = mybir.AluOpType.add
SUB = mybir.AluOpType.subtract
EXPF = mybir.ActivationFunctionType.Exp
SQRTF = mybir.ActivationFunctionType.Sqrt
SQF = mybir.ActivationFunctionType.Square
AXX = mybir.AxisListType.X

_PROGRAM_CACHE = {}


def _build_program(debug_taps=False):
    nc = bacc.Bacc(None, target_bir_lowering=False, debug=True)

    # ---- DRAM I/O ----
    xT8 = nc.dram_tensor("xT8", [X8_N, D, X8_CHUNK], BF16, kind="ExternalInput")
    xkv = nc.dram_tensor("xkv", [2, D, X8_CHUNK], BF16, kind="ExternalInput")
    w_a = nc.dram_tensor("w_a", [D, A_W], BF16, kind="ExternalInput")
    w_kd = nc.dram_tensor("w_kd", [D, LAT], BF16, kind="ExternalInput")
    w_up = nc.dram_tensor("w_up", [LAT, KV_W], BF16, kind="ExternalInput")
    w_p = nc.dram_tensor("w_p", [DLOC, D], BF16, kind="ExternalInput")
    cos4 = nc.dram_tensor("cos4", [S, H_LOC, ROPE // 2], BF16, kind="ExternalInput")
    sin4 = nc.dram_tensor("sin4", [S, H_LOC, ROPE // 2], BF16, kind="ExternalInput")
    masks = nc.dram_tensor("masks", [128, 4, QB], BF16, kind="ExternalInput")
    gain12 = nc.dram_tensor("gain12", [128, 12], F32, kind="ExternalInput")
    ones_in = nc.dram_tensor("ones_in", [128, 1], BF16, kind="ExternalInput")
    ident_in = nc.dram_tensor("ident_in", [128, 128], BF16, kind="ExternalInput")
    out = nc.dram_tensor("out", [S, D], BF16, kind="ExternalOutput")
    if debug_taps:
        dbg_ckvT = nc.dram_tensor("dbg_ckvT", [128, LT_N, S], BF16, kind="ExternalOutput")
        dbg_QT = nc.dram_tensor("dbg_QT", [128, H_LOC, S], BF16, kind="ExternalOutput")
        dbg_KT = nc.dram_tensor("dbg_KT", [128, H_LOC, S], BF16, kind="ExternalOutput")
        dbg_V = nc.dram_tensor("dbg_V", [128, ST_N, H_LOC * HD], BF16, kind="ExternalOutput")

    with tile.TileContext(nc) as tc, ExitStack() as top:
        const = top.enter_context(tc.tile_pool(name="const", bufs=1))
        big = top.enter_context(tc.tile_pool(name="big", bufs=1))

        # --- resident weights/constants, load order = consumption order ---
        wkd_sbs = [const.tile([128, 4, LT_N, 128], BF16, name=f"wkd{i}")
                   for i in range(4)]
        wkd_r = w_kd[:].rearrange("(k p) (l q) -> p k l q", p=128, q=128)
        wa_sb = const.tile([128, KT_N, A_W], BF16)
        wa_r = w_a[:].rearrange("(k p) n -> p k n", p=128)
        wup_sb = const.tile([128, LT_N, KV_W], BF16)
        cos_sb = const.tile([128, ST_N, H_LOC, ROPE // 2], BF16)
        sin_sb = const.tile([128, ST_N, H_LOC, ROPE // 2], BF16)
        gain_sb = const.tile([128, 12], F32)
        ones_sb = const.tile([128, 1], BF16)
        ident_sb = const.tile([128, 128], BF16)
        eps_sb = const.tile([128, 1], F32)
        # loaded late (only needed by attention/proj phase)
        mask_sb = const.tile([128, 4, QB], BF16)
        wp_sb = const.tile([128, H_LOC, D], BF16)

        # --- persistent activations (head-dim-major) ---
        QT = big.tile([128, H_LOC, S], BF16)   # [d, h, q]
        KT = big.tile([128, H_LOC, S], BF16)   # [d, h, k] (0:64 nope, 64:128 rope)
        V = big.tile([128, ST_N, H_LOC * HD], BF16)  # [s%128, s//128, d_loc]
        ckvT = big.tile([128, LT_N, S], BF16)  # [lat%128, lat//128, s]
        kbuf = big.tile([128, ST_N, H_LOC, HD], BF16)  # [s%128, ST, h, nope||rope]

        # ===== phase 1 scratch =====
        p1 = ExitStack()
        x8p = p1.enter_context(tc.tile_pool(name="x8p", bufs=2))
        scr = p1.enter_context(tc.tile_pool(name="scr", bufs=2))
        jnk = p1.enter_context(tc.tile_pool(name="jnk", bufs=2))

        for kt in range(4):
            nc.sync.dma_start(out=wkd_sbs[0][:, kt], in_=wkd_r[:, kt])
        xq0 = x8p.tile([128, KT_N, X8_CHUNK], BF16, tag="x8")
        xT80 = xT8[0].rearrange("(k p) s -> p k s", p=128)
        for k4 in range(0, KT_N, 4):
            nc.sync.dma_start(out=xq0[:, k4:k4 + 4, :], in_=xT80[:, k4:k4 + 4, :])
        for kt in range(4, KT_N):
            nc.sync.dma_start(out=wkd_sbs[kt // 4][:, kt % 4], in_=wkd_r[:, kt])
        for kt in range(KT_N):
            nc.sync.dma_start(out=wa_sb[:, kt, :], in_=wa_r[:, kt, :])
        nc.sync.dma_start(out=wup_sb[:], in_=w_up[:].rearrange("(k p) n -> p k n", p=128))
        nc.sync.dma_start(out=cos_sb[:], in_=cos4[:].rearrange("(t p) h f -> p t h f", p=128))
        nc.sync.dma_start(out=sin_sb[:], in_=sin4[:].rearrange("(t p) h f -> p t h f", p=128))
        nc.sync.dma_start(out=gain_sb[:], in_=gain12[:])
        nc.sync.dma_start(out=ones_sb[:], in_=ones_in[:])
        nc.sync.dma_start(out=ident_sb[:], in_=ident_in[:])
        nc.vector.memset(eps_sb[:], EPS)

        def rsqrt_act(dst, src, n, bias):
            """dst = 1/sqrt(src/n + eps): ACT Sqrt then fast DVE reciprocal."""
            nc.scalar.activation(dst, src, SQRTF, scale=1.0 / n, bias=bias)
            nc.vector.reciprocal_approx_fast(out=dst, in_=dst)

        def rope_rot(view, cos_ap, sin_ap, out1, out2):
            """out1 = x1*c + x2*s ; out2 = x2*c - x1*s (batched over heads).

            view: [128, H_LOC, 64] (normalized rope input; may alias out1/out2)
            """
            RH = ROPE // 2
            x1 = view[:, :, 0:RH]
            x2 = view[:, :, RH:ROPE]
            t1 = scr.tile([128, H_LOC, RH], F32, tag="t1")
            t2 = scr.tile([128, H_LOC, RH], F32, tag="t2")
            t3 = scr.tile([128, H_LOC, RH], F32, tag="t3")
            t4 = scr.tile([128, H_LOC, RH], F32, tag="t4")
            nc.vector.tensor_tensor(t1[:], x1, cos_ap, MULT)
            nc.vector.tensor_tensor(t2[:], x2, sin_ap, MULT)
            nc.vector.tensor_tensor(t3[:], x2, cos_ap, MULT)
            nc.vector.tensor_tensor(t4[:], x1, sin_ap, MULT)
            nc.vector.tensor_tensor(out1, t1[:], t2[:], ADD)
            nc.vector.tensor_tensor(out2, t3[:], t4[:], SUB)

        # ========== phase 1: kv_down^T + GEMM-A + norms + rope + kv_up =====
        with (
            tc.tile_pool(name="psA", bufs=2, space="PSUM") as psA,
            tc.tile_pool(name="psKD", bufs=1, space="PSUM") as psKD,
            tc.tile_pool(name="psT", bufs=3, space="PSUM") as psT,
            tc.tile_pool(name="psLQ", bufs=1, space="PSUM") as psLQ,
        ):
            def kvd_pass(p2, xq, ec0):
                kdp = psKD.tile([128, 2, 512], F32, tag="KD")
                for kt in range(KT_N):
                    for l2 in range(2):
                        nc.tensor.matmul(
                            kdp[:, l2, 0:X8_CHUNK],
                            wkd_sbs[kt // 4][:, kt % 4, 2 * p2 + l2, :],
                            xq[:, kt, :],
                            start=(kt == 0), stop=(kt == KT_N - 1))
                nc.scalar.copy(
                    ckvT[:, 2 * p2:2 * p2 + 2, ec0:ec0 + X8_CHUNK],
                    kdp[:, :, 0:X8_CHUNK])

            def ckv_norm(ec0):
                """rms-normalize this chunk's raw ckvT in place: squares on
                ACT, lt-fold on DVE, partition reduce via a ones-matmul,
                then gpsimd broadcast of the per-column scale."""
                ckv_e = ckvT[:, :, ec0:ec0 + X8_CHUNK]
                sqkd = jnk.tile([128, LT_N, X8_CHUNK], BF16, tag="sqkd")
                nc.scalar.activation(sqkd[:], ckv_e, SQF)
                ssum = scr.tile([128, X8_CHUNK], BF16, tag="ssum")
                with nc.allow_low_precision(reason="4-way lt fold of bf16 squares"):
                    nc.vector.tensor_reduce(
                        ssum[:], sqkd[:].rearrange("p l s -> p s l"), AXX, ADD)
                lsq = psLQ.tile([1, X8_CHUNK], F32, tag="lsq")
                nc.tensor.matmul(lsq[:], ones_sb[:], ssum[:],
                                 start=True, stop=True)
                rsq = scr.tile([1, X8_CHUNK], F32, tag="rsq")
                nc.scalar.activation(rsq[:], lsq[:], SQRTF,
                                     scale=1.0 / LAT, bias=eps_sb[0:1])
                nc.vector.reciprocal_approx_fast(out=rsq[:], in_=rsq[:])
                rbcs = scr.tile([128, X8_CHUNK], F32, tag="rbcs")
                nc.gpsimd.partition_broadcast(rbcs[:], rsq[:])
                for lt in range(LT_N):
                    nc.vector.tensor_tensor(
                        ckvT[:, lt, ec0:ec0 + X8_CHUNK],
                        ckvT[:, lt, ec0:ec0 + X8_CHUNK], rbcs[:], MULT)

            def gemma_part(ST, xs):
                s0 = ST * 128
                aq = psA.tile([128, 512], F32, tag="A")
                ak = psA.tile([128, 512], F32, tag="A")
                for kt in range(KT_N):
                    nc.tensor.matmul(
                        aq[:], xs[:, kt, :], wa_sb[:, kt, 0:512],
                        start=(kt == 0), stop=(kt == KT_N - 1))
                    nc.tensor.matmul(
                        ak[:, 0:256], xs[:, kt, :], wa_sb[:, kt, 512:768],
                        start=(kt == 0), stop=(kt == KT_N - 1))

                # ---- evict + batched stats ----
                asb = scr.tile([128, A_W], F32, tag="asb")
                nc.scalar.copy(asb[:, 0:512], aq[:])
                nc.scalar.copy(asb[:, 512:768], ak[:, 0:256])
                junk = jnk.tile([128, A_W], BF16, tag="junk")
                nc.scalar.activation(junk[:, 0:512], aq[:], SQF)
                nc.scalar.activation(junk[:, 512:768], ak[:, 0:256], SQF)
                rs12 = scr.tile([128, 12], F32, tag="rs12")
                nc.vector.tensor_reduce(
                    rs12[:],
                    junk[:].rearrange("p (g c) -> p g c", c=64),
                    AXX, ADD)
                rsqrt_act(rs12[:], rs12[:], 64, eps_sb[:])
                nc.vector.tensor_tensor(rs12[:], rs12[:], gain_sb[:], MULT)

                # ---- apply norms (q+krope, 12 groups of 64) ----
                nrm = scr.tile([128, A_W], BF16, tag="nrm")
                nc.vector.tensor_tensor(
                    nrm[:].rearrange("p (g c) -> p g c", c=64),
                    asb[:].rearrange("p (g c) -> p g c", c=64),
                    rs12[:].to_broadcast([128, 12, 64]), MULT)

                # ---- rope (in-place on nrm / into kbuf) ----
                RH = ROPE // 2
                nrm_q = nrm[:, 0:A_QW].rearrange(
                    "p (h t c) -> p h t c", t=2, c=64)
                qro = nrm_q[:, :, 1, :]
                rope_rot(qro, cos_sb[:, ST], sin_sb[:, ST],
                         qro[:, :, 0:RH], qro[:, :, RH:ROPE])
                kro = nrm[:, A_QW:A_W].rearrange("p (h c) -> p h c", c=64)
                rope_rot(kro, cos_sb[:, ST], sin_sb[:, ST],
                         kbuf[:, ST, :, NOPE:NOPE + RH],
                         kbuf[:, ST, :, NOPE + RH:HD])

                return ST, nrm

            def gemma_tr(ST, nrm):
                # q transposes (nope||rope in one shot per head); emitted
                # after later matmuls so the PE never waits on the norm/rope
                # vector chain
                s0 = ST * 128
                for h in range(H_LOC):
                    tq = psT.tile([128, 128], BF16, tag="tq")
                    nc.tensor.transpose(
                        tq[:], nrm[:, h * HD:(h + 1) * HD], ident_sb[:])
                    nc.scalar.copy(QT[:, h, s0:s0 + 128], tq[:])

            def g3_part(ST):
                s0 = ST * 128
                # (shares the psA ring: same shapes, evicted promptly)
                kv1 = psA.tile([128, 512], F32, tag="A")
                kv2 = psA.tile([128, 512], F32, tag="A")
                for lt in range(LT_N):
                    lhs = ckvT[:, lt, s0:s0 + 128]
                    nc.tensor.matmul(
                        kv1[:], lhs, wup_sb[:, lt, 0:512],
                        start=(lt == 0), stop=(lt == LT_N - 1))
                    nc.tensor.matmul(
                        kv2[:, 0:256], lhs, wup_sb[:, lt, 512:768],
                        start=(lt == 0), stop=(lt == LT_N - 1))
                # k_nope batched norm straight into kbuf[:, ST, :, 0:64]
                kvev = scr.tile([128, H_LOC * NOPE], F32, tag="kvev")
                nc.scalar.copy(kvev[:], kv1[:, 0:H_LOC * NOPE])
                junkk = jnk.tile([128, H_LOC * NOPE], BF16, tag="junkk")
                nc.scalar.activation(junkk[:], kv1[:, 0:H_LOC * NOPE], SQF)
                rsk = scr.tile([128, H_LOC], F32, tag="rsk")
                nc.vector.tensor_reduce(
                    rsk[:], junkk[:].rearrange("p (g c) -> p g c", c=64),
                    AXX, ADD)
                rsqrt_act(rsk[:], rsk[:], 64, eps_sb[:])
                nc.vector.tensor_tensor(
                    kbuf[:, ST, :, 0:NOPE],
                    kvev[:].rearrange("p (g c) -> p g c", c=64),
                    rsk[:].to_broadcast([128, H_LOC, 64]), MULT)
                # V evict (split across the two chain tiles)
                nc.scalar.copy(V[:, ST, 0:256], kv1[:, 256:512])
                nc.scalar.copy(V[:, ST, 256:512], kv2[:, 0:256])
                return ST

            def g3_tr(ST):
                # k transposes (nope||rope in one shot per head), deferred
                # past later matmuls to cover the k_nope norm chain
                s0 = ST * 128
                for h in range(H_LOC):
                    tk = psT.tile([128, 128], BF16, tag="tq")
                    nc.tensor.transpose(tk[:], kbuf[:, ST, h, :], ident_sb[:])
                    nc.vector.tensor_copy(KT[:, h, s0:s0 + 128], tk[:])

            _p1sc = nc.enter_named_scope("p1", False)[0]
            for e in range(X8_N):
                if e == 0:
                    xq = xq0
                else:
                    xq = x8p.tile([128, KT_N, X8_CHUNK], BF16, tag="x8")
                    nc.sync.dma_start(
                        out=xq[:], in_=xT8[e].rearrange("(k p) s -> p k s", p=128))
                ec0 = e * X8_CHUNK
                # every vector chain (norm/rope, latent norm, k_nope norm) is
                # covered by the next block of independent matmuls before any
                # transpose that consumes it is issued
                kvd_pass(0, xq, ec0)
                gm0 = gemma_part(2 * e, xq[:, :, 0:128])
                kvd_pass(1, xq, ec0)
                gemma_tr(*gm0)
                ckv_norm(ec0)
                gm1 = gemma_part(2 * e + 1, xq[:, :, 128:256])
                g30 = g3_part(2 * e)
                gemma_tr(*gm1)
                g31 = g3_part(2 * e + 1)
                g3_tr(g30)
                g3_tr(g31)
            nc.leave_named_scope("p1", _p1sc, False)

        p1.close()

        if debug_taps:
            nc.sync.dma_start(out=dbg_ckvT[:], in_=ckvT[:])
            nc.sync.dma_start(out=dbg_QT[:], in_=QT[:])
            nc.sync.dma_start(out=dbg_KT[:], in_=KT[:])
            nc.sync.dma_start(out=dbg_V[:], in_=V[:])

        # late const loads (attention/proj only)
        nc.sync.dma_start(out=mask_sb[:], in_=masks[:])
        nc.sync.dma_start(out=wp_sb[:], in_=w_p[:].rearrange("(k p) n -> p k n", p=128))

        # ====== phase 3: attention + out projection (fused per q-block) ======
        # Software-pipelined k-loop over PAIRS of 128-k-tiles: one exp per
        # pair (halves ACT instruction+access overhead), scores of pair n+1
        # emitted before V-matmuls of pair n so the PE never waits on exp.
        # Softmax denominator: P accumulated pairwise on DVE, reduced over
        # partitions by gpsimd (no PE ones-matmuls, no PSUM bank).
        inv_sqrt_hd = 1.0 / math.sqrt(HD)
        with (
            tc.tile_pool(name="pP", bufs=6) as pP,
            tc.tile_pool(name="pAcc", bufs=3) as pAcc,
            tc.tile_pool(name="pL", bufs=2) as pL,
            tc.tile_pool(name="pRb", bufs=2) as pRb,
            tc.tile_pool(name="pY", bufs=2) as pY,
            tc.tile_pool(name="pO", bufs=4) as pO,
            tc.tile_pool(name="psS", bufs=2, space="PSUM") as psS,
            tc.tile_pool(name="psY", bufs=2, space="PSUM") as psY,
            tc.tile_pool(name="psL", bufs=2, space="PSUM") as psL,
        ):
            def emit_proj_sub(j, yT, sub):
                # accumulators borrow the score-pair ring (never open at the
                # same time as score chains; only lane 0's bank is used)
                q0 = j * QB
                if True:
                    sq0 = q0 + sub * 128
                    for nb in range(D // NB):
                        opst = psS.tile([128, 2, QB], F32, tag="S")
                        ops = opst[:, 0, :]
                        for h in range(H_LOC):
                            nc.tensor.matmul(
                                ops, yT[:, h, sub * 128:(sub + 1) * 128],
                                wp_sb[:, h, nb * NB:(nb + 1) * NB],
                                start=(h == 0), stop=(h == H_LOC - 1))
                        osb = pO.tile([128, NB], BF16, tag="osb")
                        if nb % 4 == 0:
                            nc.scalar.copy(osb[:], ops)
                        else:
                            nc.vector.tensor_copy(osb[:], ops)
                        nc.sync.dma_start(
                            out=out[sq0:sq0 + 128, nb * NB:(nb + 1) * NB],
                            in_=osb[:])

            _p3sc = nc.enter_named_scope("p3", False)[0]
            prev_proj = None
            for j in range(S // QB):
                q0 = j * QB
                nkt = (q0 + QB) // 128
                npair = nkt // 2
                dpair0 = (q0 // 128) // 2  # first diagonal pair index
                yT = pY.tile([128, H_LOC, QB], BF16, tag="yT")

                def fin_head(h, yT, yps, pacc):
                    # softmax denominator + normalize, deferred into the next
                    # head's k-loop so the PE never waits on the pacc chain
                    lps = psL.tile([1, QB], F32, tag="L")
                    for i in range(2):
                        nc.tensor.matmul(
                            lps[:], ones_sb[:], pacc[:, i, :],
                            start=(i == 0), stop=(i == 1))
                    r = pL.tile([1, QB], F32, tag="r")
                    nc.vector.reciprocal_approx_fast(out=r[:], in_=lps[:])
                    rbc = pRb.tile([128, QB], F32, tag="rbc")
                    nc.gpsimd.partition_broadcast(rbc[:], r[:])
                    nc.vector.tensor_tensor(
                        yT[:, h, :], yps[:], rbc[:], MULT)

                def emit_v(prev):
                    # V-matmuls for a (head, pair) item, one pipeline step
                    # behind its scores/exp so the PE never waits on ACT
                    ph, pyps, pkp, pP_t = prev
                    for i in range(2):
                        kt = 2 * pkp + i
                        nc.tensor.matmul(
                            pyps[:], V[:, kt, ph * HD:(ph + 1) * HD],
                            pP_t[:, i, :],
                            start=(kt == 0), stop=(kt == nkt - 1))

                pfin = None
                prev = None
                for h in range(H_LOC):
                    yps = psY.tile([128, QB], F32, tag="Y")
                    pacc = pAcc.tile([128, 2, QB], BF16, tag="acc")
                    for kp in range(npair):
                        if kp == 1 and pfin is not None:
                            fin_head(*pfin)
                            pfin = None
                        spair = psS.tile([128, 2, QB], F32, tag="S")
                        for i in range(2):
                            kt = 2 * kp + i
                            nc.tensor.matmul(
                                spair[:, i, :],
                                KT[:, h, kt * 128:(kt + 1) * 128],
                                QT[:, h, q0:q0 + QB],
                                start=True, stop=True)
                        Pp = pP.tile([128, 2, QB], BF16, tag="P")
                        nc.scalar.activation(
                            Pp[:], spair[:], EXPF, scale=inv_sqrt_hd)
                        if kp >= dpair0:
                            di = 2 * (kp - dpair0)
                            nc.vector.tensor_tensor(
                                Pp[:], Pp[:], mask_sb[:, di:di + 2, :],
                                MULT)
                        if kp == 0:
                            nc.vector.tensor_copy(pacc[:], Pp[:])
                        else:
                            nc.vector.tensor_tensor(
                                pacc[:], pacc[:], Pp[:], ADD)
                        if prev is not None:
                            emit_v(prev)
                        prev = (h, yps, kp, Pp)
                    pfin = (h, yT, yps, pacc)
                    # deferred projection: one quarter of the previous j's
                    # proj after each head, spreading its PE work across
                    # this j's ACT-bound attention
                    if prev_proj is not None:
                        emit_proj_sub(prev_proj[0], prev_proj[1], h)

                emit_v(prev)
                prev = None
                fin_head(*pfin)
                prev_proj = (j, yT)
            for sub in range(QB // 128):
                emit_proj_sub(prev_proj[0], prev_proj[1], sub)
            nc.leave_named_scope("p3", _p3sc, False)
    nc.compile()
    return nc


def _prep_inputs(x, w_q_krope, w_kv_down, w_kv_up, w_proj, q_gain):
    """Build the 8 per-core input maps (host-side sharding)."""
    inv_freq = ROPE_BASE ** (-np.arange(0, ROPE, 2, dtype=np.float32) / ROPE)
    t = np.arange(S, dtype=np.float32)
    freqs = np.outer(t, inv_freq)                      # (S, 32)
    cos4 = np.ascontiguousarray(np.broadcast_to(
        np.cos(freqs)[:, None, :], (S, H_LOC, ROPE // 2))).astype(BF)
    sin4 = np.ascontiguousarray(np.broadcast_to(
        np.sin(freqs)[:, None, :], (S, H_LOC, ROPE // 2))).astype(BF)

    kk = np.arange(128)[:, None, None]
    dd = np.arange(4)[None, :, None]
    qq = np.arange(QB)[None, None, :]
    masks = (kk + 128 * dd <= qq).astype(BF)           # [128, 4, QB]

    ones_in = np.ones((128, 1), dtype=BF)
    ident_in = np.eye(128, dtype=np.float32).astype(BF)

    # x^T per batch, chunked: [X8_N, D, X8_CHUNK]
    xT_chunks = []
    for b in range(B):
        xT = np.ascontiguousarray(x[b].T).astype(BF)   # [D, S]
        xT_chunks.append(np.ascontiguousarray(
            xT.reshape(D, X8_N, X8_CHUNK).transpose(1, 0, 2)))

    w_kd = np.ascontiguousarray(w_kv_down).astype(BF)  # [D, LAT]

    in_maps = []
    for c in range(N_CORES):
        b = c // H_LOC
        hg = c % H_LOC
        heads = [hg * H_LOC + i for i in range(H_LOC)]
        w_a = np.concatenate(
            [w_q_krope[:, h * HD:(h + 1) * HD] for h in heads]
            + [w_q_krope[:, D + h * ROPE:D + (h + 1) * ROPE] for h in heads],
            axis=1).astype(BF)                          # [D, 768]
        w_up = np.concatenate(
            [w_kv_up[:, h * NOPE:(h + 1) * NOPE] for h in heads]
            + [w_kv_up[:, NOPE * H + h * HD:NOPE * H + (h + 1) * HD]
               for h in heads], axis=1).astype(BF)      # [LAT, 768]
        w_p = w_proj[hg * DLOC:(hg + 1) * DLOC, :].astype(BF)   # [512, D]
        g = q_gain[heads].astype(np.float32)
        g12 = np.concatenate([np.repeat(g, 2), np.ones(4, np.float32)])
        gain12 = np.ascontiguousarray(
            np.broadcast_to(g12[None, :], (128, 12))).astype(np.float32)
        # this core's group-rank quarter of x^T for the latent path
        xkv = np.ascontiguousarray(xT_chunks[b][2 * hg:2 * hg + 2])
        in_maps.append({
            "xT8": xT_chunks[b],
            "xkv": xkv,
            "w_a": np.ascontiguousarray(w_a),
            "w_kd": w_kd,
            "w_up": np.ascontiguousarray(w_up),
            "w_p": np.ascontiguousarray(w_p),
            "cos4": cos4, "sin4": sin4, "masks": masks,
            "gain12": gain12,
            "ones_in": ones_in, "ident_in": ident_in,
        })
    return in_maps


def kernel(x, w_q_krope, w_kv_down, w_kv_up, w_proj, q_gain, **_unused):
    x = np.asarray(x, dtype=np.float32)
    w_q_krope = np.asarray(w_q_krope, dtype=np.float32)
    w_kv_down = np.asarray(w_kv_down, dtype=np.float32)
    w_kv_up = np.asarray(w_kv_up, dtype=np.float32)
    w_proj = np.asarray(w_proj, dtype=np.float32)
    q_gain = np.asarray(q_gain, dtype=np.float32)

    if "nc" not in _PROGRAM_CACHE:
        _PROGRAM_CACHE["nc"] = _build_program()
    nc = _PROGRAM_CACHE["nc"]

    in_maps = _prep_inputs(x, w_q_krope, w_kv_down, w_kv_up, w_proj, q_gain)
    res = run_bass_kernel_spmd(nc, in_maps, list(range(N_CORES)))

    out = np.zeros((B, S, D), dtype=np.float32)
    for c in range(N_CORES):
        out[c // H_LOC] += res.results[c]["out"]
    return out
